# revision 40
# baseline (speedup 1.0000x reference)
"""CIKA conv block on 8 Trainium2 NeuronCores.

Sharding: pure data parallel. 8 shards = (batch n, H half). Each core gets a
zero-padded, W-strip-interleaved bf16 slice of `lower`/`upper` plus
replicated (host-preprocessed) weights, and computes its (32, 128, 256)
slice of both outputs (low, up).

On-chip layout: [128 partitions = 4 W-strips x 32 channels].  Depthwise 5x5
convs run on the TensorEngine as 25 diagonal-matmul taps accumulated in PSUM
(spatial shifts are free AP offsets into the padded SBUF plane).  1x1 convs
are block-diagonal matmuls (kron(I_strips, W^T)).  The dynamic (involution)
conv: a selector matmul replicates each KSA tap plane across the 32 channel
partitions into PSUM; one fused DVE scalar_tensor_tensor forms
m_t = (x_shift * w[c,t]) * ksa_rep in bf16; the tap sum and the following
1x1 are folded into one PSUM accumulation of W_low @ m_t over the 25 taps.
All matmul operands are bf16 (enables PE fast-weight-load); PSUM
accumulation stays fp32.
"""

import os
from contextlib import ExitStack

import numpy as np

import bass_rust
import concourse.bacc as bacc
import concourse.bass as bass
import concourse.mybir as mybir
import concourse.tile as tile
from concourse.bass_utils import run_bass_kernel_spmd

F32 = mybir.dt.float32
BF16 = mybir.dt.bfloat16
F16 = mybir.dt.float16
F8 = mybir.dt.float8e4
DR = mybir.MatmulPerfMode.DoubleRow
AF = mybir.ActivationFunctionType
ALU = mybir.AluOpType

KK = 5          # kernel size
CH = 32         # channels
NB, H, W = 4, 256, 256
N_CORES = 8
HSH = H // 2    # rows per core (one batch-half per core)
ROWS_T = 32     # output rows per on-chip tile
NT = HSH // ROWS_T
SW = 64         # strip width (W / 4)
TAPS = [(i, j) for i in range(KK) for j in range(KK)]

LAST_EXEC_NS = None


def _emit(ctx: ExitStack, tc: tile.TileContext, io):
    nc = tc.nc
    (lower_d, upper_d, lower8_d, upper8_d, wdw_d, wdw8_d, sel_d, wm1_d,
     wm2_d, wk1_d, wk2_d, wlow_d, wup_d, wdyn_d, bias_d, low_od, up_od) = io

    wpool = ctx.enter_context(tc.tile_pool(name="wts", bufs=1))
    inp = ctx.enter_context(tc.tile_pool(name="inp", bufs=2))
    work = ctx.enter_context(tc.tile_pool(name="work", bufs=2))
    outp = ctx.enter_context(tc.tile_pool(name="outp", bufs=2))
    ps_dw = ctx.enter_context(tc.tile_pool(name="psdw", bufs=2, space="PSUM"))
    ps_pw = ctx.enter_context(tc.tile_pool(name="pspw", bufs=2, space="PSUM"))
    ps_rep = ctx.enter_context(tc.tile_pool(name="psrep", bufs=2,
                                            space="PSUM"))

    # ---- DMA order tuned so tile-0 gate-conv can start ASAP ----
    # 1) tile-0 fp8 inputs + the weights the kca chain needs first
    low8_0 = inp.tile([128, ROWS_T + 5, SW + 4], F8, tag="low8_in")
    up8_0 = inp.tile([128, ROWS_T + 5, SW + 4], F8, tag="up8_in")
    nc.sync.dma_start(low8_0[:], lower8_d[:, 0:ROWS_T + 5, :])
    nc.sync.dma_start(up8_0[:], upper8_d[:, 0:ROWS_T + 5, :])
    # fp8 DoubleRow tap-pair weights for the two gate-path dw5 convs.
    # Vertical pairs (k-tile delta = row stride): per col j, row pairs
    # (0,1), (2,3), (4,zero) -> 15 pairs. [128, cv*30 + pair*2 + kt, 128]
    w_dw8 = wpool.tile([128, 60, 128], F8)
    nc.sync.dma_start(w_dw8[:], wdw8_d[:])
    w_m1 = wpool.tile([128, 32], BF16)
    nc.sync.dma_start(w_m1[:], wm1_d[:])
    w_m2 = wpool.tile([32, 128], BF16)
    nc.sync.dma_start(w_m2[:], wm2_d[:])
    bias = wpool.tile([128, 9], F32)
    nc.sync.dma_start(bias[:], bias_d[:])
    # 2) tile-0 bf16 inputs (needed mid-tile by the STT / up branch)
    low_0 = inp.tile([128, ROWS_T + 4, SW + 4], BF16, tag="low_in")
    up_0 = inp.tile([128, ROWS_T + 4, SW + 4], BF16, tag="up_in")
    nc.sync.dma_start(low_0[:], lower_d[:, 0:ROWS_T + 4, :])
    nc.sync.dma_start(up_0[:], upper_d[:, 0:ROWS_T + 4, :])
    # 3) remaining weights in order of first use
    w_k1 = wpool.tile([128, 100], BF16)
    nc.sync.dma_start(w_k1[:], wk1_d[:])
    w_k2 = wpool.tile([100, 64], BF16)
    nc.sync.dma_start(w_k2[:], wk2_d[:])
    sel = wpool.tile([128, 25, 128], BF16)
    nc.sync.dma_start(sel[:], sel_d[:])
    w_low = wpool.tile([128, 128], BF16)
    nc.sync.dma_start(w_low[:], wlow_d[:])
    wdyn = wpool.tile([128, 25], F32)
    nc.sync.dma_start(wdyn[:], wdyn_d[:])
    # w_dw holds only the up-branch dw5 taps (bf16 diagonal per tap)
    w_dw = wpool.tile([128, 25, 128], BF16)
    nc.sync.dma_start(w_dw[:], wdw_d[:])
    w_up = wpool.tile([128, 128], BF16)
    nc.sync.dma_start(w_up[:], wup_d[:])

    def bcol(idx, p=128):
        return bias[0:p, idx:idx + 1]

    # PE can encode only one sync wait per matmul (LDWEIGHTS struct limit).
    # Warm-up matmuls make PE observe every weight-DMA queue once, so real
    # matmuls transitively need no weight waits — just their rhs producer.
    # Split into two groups so the first real matmuls only wait on the
    # early (small) weight transfers.
    sc = ps_pw.tile([1, 1], F32, tag="pspw")
    for wap in (w_dw8[0:1, 0, 0:1], w_m1[0:1, 0:1], w_m2[0:1, 0:1]):
        nc.tensor.matmul(sc[:], wap, wap, start=True, stop=True)

    def late_warmups():
        sc2 = ps_pw.tile([1, 1], F32, tag="pspw")
        for wap in (w_k1[0:1, 0:1], w_k2[0:1, 0:1], sel[0:1, 0, 0:1],
                    w_low[0:1, 0:1], w_dw[0:1, 0, 0:1], w_up[0:1, 0:1]):
            nc.tensor.matmul(sc2[:], wap, wap, start=True, stop=True)

    for it in range(NT):
        r0 = it * ROWS_T
        if it == 0:
            low_t, up_t, low8_t, up8_t = low_0, up_0, low8_0, up8_0
        else:
            low_t = inp.tile([128, ROWS_T + 4, SW + 4], BF16, tag="low_in")
            up_t = inp.tile([128, ROWS_T + 4, SW + 4], BF16, tag="up_in")
            low8_t = inp.tile([128, ROWS_T + 5, SW + 4], F8, tag="low8_in")
            up8_t = inp.tile([128, ROWS_T + 5, SW + 4], F8, tag="up8_in")
            # shards pre-striped on the host to [128 = strip*32+c, rows, 68]
            nc.sync.dma_start(low_t[:], lower_d[:, r0:r0 + ROWS_T + 4, :])
            nc.sync.dma_start(up_t[:], upper_d[:, r0:r0 + ROWS_T + 4, :])
            nc.sync.dma_start(low8_t[:], lower8_d[:, r0:r0 + ROWS_T + 5, :])
            nc.sync.dma_start(up8_t[:], upper8_d[:, r0:r0 + ROWS_T + 5, :])

        def dr_rhs(src8, q, i0, j):
            # [128, 2 (vertical tap-pair k-tiles, delta = row stride), 8, 64]
            base = src8[:, q * 8 + i0:q * 8 + i0 + 8, j:j + SW]
            raw = [list(d) for d in base.ap]
            return bass_rust.AP(
                base.tensor, base.offset,
                [raw[0], [SW + 4, 2], raw[1], raw[2]])

        # gate-path dw5: 15 fp8 DoubleRow vertical tap-pairs per q-chunk
        # (per col j: row pairs (0,1), (2,3), (4,zero))
        def dw5_dr(src8, cv, out_sb, bias_idx):
            for q in range(4):
                ps = ps_dw.tile([128, 8, SW], F32, tag="psdw")
                pp = 0
                for j in range(5):
                    for i0 in (0, 2, 4):
                        w8 = cv * 30 + pp * 2
                        nc.tensor.matmul(
                            ps[:], w_dw8[:, w8:w8 + 2, :],
                            dr_rhs(src8, q, i0, j),
                            start=(pp == 0), stop=(pp == 14), perf_mode=DR)
                        pp += 1
                nc.scalar.activation(out_sb[:, q * 8:(q + 1) * 8, :], ps[:],
                                     AF.Relu, bias=bcol(bias_idx))

        t_kca = work.tile([128, ROWS_T, SW], BF16, tag="t_kca")
        dw5_dr(low8_t, 0, t_kca, 0)
        if it == 0:
            late_warmups()

        # ---- KCA chain: 1x1 (32->8) relu, 1x1 (8->32) sigmoid ----
        m1o = work.tile([32, ROWS_T, SW], BF16, tag="m1o")
        for q in range(4):
            ps = ps_pw.tile([32, 8, SW], F32, tag="pspw")
            nc.tensor.matmul(ps[:], w_m1[:], t_kca[:, q * 8:(q + 1) * 8, :],
                             start=True, stop=True)
            nc.scalar.activation(m1o[:, q * 8:(q + 1) * 8, :], ps[:],
                                 AF.Relu, bias=bcol(3, 32))
        kca = work.tile([128, ROWS_T, SW], BF16, tag="kca")
        for q in range(4):
            ps = ps_pw.tile([128, 8, SW], F32, tag="pspw")
            nc.tensor.matmul(ps[:], w_m2[:], m1o[:, q * 8:(q + 1) * 8, :],
                             start=True, stop=True)
            nc.scalar.activation(kca[:, q * 8:(q + 1) * 8, :], ps[:],
                                 AF.Sigmoid, bias=bcol(4))

        # ---- KSA chain (strip pairs: K=64 -> M=100, then K=100 -> M=64) ----
        t_ksa = work.tile([128, ROWS_T, SW], BF16, tag="t_ksa")
        dw5_dr(up8_t, 1, t_ksa, 1)
        k1o = work.tile([100, 2, ROWS_T, SW], BF16, tag="k1o")
        for g in range(2):
            for q in range(4):
                ps = ps_pw.tile([100, 8, SW], F32, tag="pspw")
                nc.tensor.matmul(
                    ps[:], w_k1[g * 64:(g + 1) * 64, :],
                    t_ksa[g * 64:(g + 1) * 64, q * 8:(q + 1) * 8, :],
                    start=True, stop=True)
                nc.scalar.activation(k1o[:, g, q * 8:(q + 1) * 8, :], ps[:],
                                     AF.Relu, bias=bcol(5, 100))
        # ksa laid out [128 = strip*32 + tap, rows, cols] (slots 25-31 pad)
        ksa = work.tile([128, ROWS_T, SW], BF16, tag="ksa")
        for g in range(2):
            for q in range(4):
                ps = ps_pw.tile([64, 8, SW], F32, tag="pspw")
                nc.tensor.matmul(ps[:], w_k2[:],
                                 k1o[:, g, q * 8:(q + 1) * 8, :],
                                 start=True, stop=True)
                nc.scalar.activation(
                    ksa[64 * g:64 * (g + 1), q * 8:(q + 1) * 8, :], ps[:],
                    AF.Sigmoid, bias=bcol(6, 64))

        # ---- dynamic conv + low 1x1 ----
        # m_t = (lower_shift * w_dyn[c,t]) * ksa_rep[t].  The tap sum runs as
        # an fp16 ping-pong add chain on DVE/GpSimd (rep spans 2 PSUM banks so
        # each STT covers 16 rows); W_low is applied once to the summed taps.
        low_o = outp.tile([128, ROWS_T, SW], BF16, tag="low_o")
        for hf in range(2):
            accs = [work.tile([128, 16, SW], F16, tag=f"acc{k}",
                              name=f"acc{k}") for k in (0, 1)]
            eng = nc.vector if (2 * it + hf) >= 5 else nc.gpsimd
            for t, (i, j) in enumerate(TAPS):
                rep = ps_rep.tile([128, 16, SW], F32, tag="rep")
                for qq in range(2):
                    q = hf * 2 + qq
                    nc.tensor.matmul(rep[:, qq * 8:(qq + 1) * 8, :],
                                     sel[:, t, :],
                                     ksa[:, q * 8:(q + 1) * 8, :],
                                     start=True, stop=True)
                if t == 0:
                    mt = accs[0]
                else:
                    mt = work.tile([128, 16, SW], BF16, tag="mt")
                nc.vector.scalar_tensor_tensor(
                    mt[:], low_t[:, hf * 16 + i:hf * 16 + i + 16, j:j + SW],
                    wdyn[:, t:t + 1], rep[:], ALU.mult, ALU.mult)
                if t > 0:
                    eng.tensor_add(accs[t % 2][:], mt[:],
                                   accs[(t + 1) % 2][:])
            for qq in range(2):
                q = hf * 2 + qq
                ps = ps_pw.tile([128, 8, SW], F32, tag="pspw")
                nc.tensor.matmul(ps[:], w_low[:],
                                 accs[0][:, qq * 8:(qq + 1) * 8, :],
                                 start=True, stop=True)
                nc.scalar.activation(low_o[:, q * 8:(q + 1) * 8, :], ps[:],
                                     AF.Identity, bias=bcol(7))
        nc.sync.dma_start(low_od[:, r0:r0 + ROWS_T, :], low_o[:])

        # ---- up branch: dw5 + bias, gate by kca (fused on DVE), 1x1 ----
        gated = work.tile([128, ROWS_T, SW], BF16, tag="gated")
        for q in range(4):
            ps = ps_dw.tile([128, 8, SW], F32, tag="psdw")
            for t, (i, j) in enumerate(TAPS):
                nc.tensor.matmul(
                    ps[:], w_dw[:, t, :],
                    up_t[:, q * 8 + i:q * 8 + i + 8, j:j + SW],
                    start=(t == 0), stop=(t == 24))
            nc.vector.scalar_tensor_tensor(
                gated[:, q * 8:(q + 1) * 8, :], ps[:], bcol(2),
                kca[:, q * 8:(q + 1) * 8, :], ALU.add, ALU.mult)
        up_o = outp.tile([128, ROWS_T, SW], BF16, tag="up_o")
        for q in range(4):
            ps = ps_pw.tile([128, 8, SW], F32, tag="pspw")
            nc.tensor.matmul(ps[:], w_up[:], gated[:, q * 8:(q + 1) * 8, :],
                             start=True, stop=True)
            nc.scalar.activation(up_o[:, q * 8:(q + 1) * 8, :], ps[:],
                                 AF.Identity, bias=bcol(8))
        nc.sync.dma_start(up_od[:, r0:r0 + ROWS_T, :], up_o[:])


_NC_CACHE = {}


def _build_nc():
    if "nc" in _NC_CACHE:
        return _NC_CACHE["nc"]
    nc = bacc.Bacc("TRN2", target_bir_lowering=False)
    lower_d = nc.dram_tensor("lower_sh", (128, HSH + 4, SW + 4), BF16,
                             kind="ExternalInput")
    upper_d = nc.dram_tensor("upper_sh", (128, HSH + 4, SW + 4), BF16,
                             kind="ExternalInput")
    lower8_d = nc.dram_tensor("lower8_sh", (128, HSH + 5, SW + 4), F8,
                              kind="ExternalInput")
    upper8_d = nc.dram_tensor("upper8_sh", (128, HSH + 5, SW + 4), F8,
                              kind="ExternalInput")
    wdw_d = nc.dram_tensor("w_dw", (128, 25, 128), BF16, kind="ExternalInput")
    wdw8_d = nc.dram_tensor("w_dw8", (128, 60, 128), F8,
                            kind="ExternalInput")
    sel_d = nc.dram_tensor("sel", (128, 25, 128), BF16, kind="ExternalInput")
    wm1_d = nc.dram_tensor("w_m1", (128, 32), BF16, kind="ExternalInput")
    wm2_d = nc.dram_tensor("w_m2", (32, 128), BF16, kind="ExternalInput")
    wk1_d = nc.dram_tensor("w_k1", (128, 100), BF16, kind="ExternalInput")
    wk2_d = nc.dram_tensor("w_k2", (100, 64), BF16, kind="ExternalInput")
    wlow_d = nc.dram_tensor("w_low", (128, 128), BF16, kind="ExternalInput")
    wup_d = nc.dram_tensor("w_up", (128, 128), BF16, kind="ExternalInput")
    wdyn_d = nc.dram_tensor("w_dyn", (128, 25), F32, kind="ExternalInput")
    bias_d = nc.dram_tensor("biases", (128, 9), F32, kind="ExternalInput")
    low_od = nc.dram_tensor("low_out", (128, HSH, SW), BF16,
                            kind="ExternalOutput")
    up_od = nc.dram_tensor("up_out", (128, HSH, SW), BF16,
                           kind="ExternalOutput")
    io = (lower_d, upper_d, lower8_d, upper8_d, wdw_d, wdw8_d, sel_d, wm1_d,
          wm2_d, wk1_d, wk2_d, wlow_d, wup_d, wdyn_d, bias_d, low_od, up_od)
    with tile.TileContext(nc) as tc:
        with ExitStack() as ctx:
            _emit(ctx, tc, io)
    nc.compile()
    _NC_CACHE["nc"] = nc
    return nc


def _prep_weights(kca_dw_w, kca_dw_b, kca_m1_w, kca_m1_b, kca_m2_w, kca_m2_b,
                  ksa_dw_w, ksa_dw_b, ksa_m1_w, ksa_m1_b, ksa_m2_w, ksa_m2_b,
                  low_dyn_w, low_dyn_b, low_pw_w, low_pw_b,
                  up_dw_w, up_dw_b, up_pw_w, up_pw_b):
    f = np.float32
    import ml_dtypes
    bf = ml_dtypes.bfloat16
    f8 = ml_dtypes.float8_e4m3
    w_dw = np.zeros((128, 25, 128), f)
    ar = np.arange(128)
    w2 = np.asarray(up_dw_w, f).reshape(CH, 25)
    for t in range(25):
        w_dw[ar, t, ar] = np.tile(w2[:, t], 4)
    # DoubleRow fp8 vertical tap-pair diagonals for kca/ksa dw5:
    # per col j, row pairs (0,1), (2,3), (4,zero)
    w_dw8 = np.zeros((128, 60, 128), f)
    for cv, wt in enumerate([kca_dw_w, ksa_dw_w]):
        w3 = np.asarray(wt, f).reshape(CH, 5, 5)  # (c, i, j)
        pp = 0
        for j in range(5):
            for i0 in (0, 2, 4):
                for kt in range(2):
                    if i0 + kt < 5:
                        w_dw8[ar, cv * 30 + pp * 2 + kt, ar] = \
                            np.tile(w3[:, i0 + kt, j], 4)
                pp += 1
    sel = np.zeros((128, 25, 128), f)
    for s in range(4):
        for t in range(25):
            sel[s * 32 + t, t, s * 32:(s + 1) * 32] = 1.0
    i4, i2 = np.eye(4, dtype=f), np.eye(2, dtype=f)
    w_m1 = np.kron(i4, np.asarray(kca_m1_w, f).T)        # (128, 32)
    w_m2 = np.kron(i4, np.asarray(kca_m2_w, f).T)        # (32, 128)
    w_k1 = np.kron(i2, np.asarray(ksa_m1_w, f).T)        # (64, 100)
    w_k1 = np.vstack([w_k1, w_k1])                       # (128, 100) dup
    w_k2 = np.zeros((100, 64), f)                        # padded to 32-slots
    w2t = np.asarray(ksa_m2_w, f).T                      # (50, 25)
    for sl in range(2):
        w_k2[sl * 50:(sl + 1) * 50, sl * 32:sl * 32 + 25] = w2t
    w_low = np.kron(i4, np.asarray(low_pw_w, f).T)       # (128, 128)
    w_up = np.kron(i4, np.asarray(up_pw_w, f).T)         # (128, 128)
    w_dyn = np.tile(np.asarray(low_dyn_w, f).reshape(CH, 25), (4, 1))
    bias = np.zeros((128, 9), f)
    bias[:, 0] = np.tile(np.asarray(kca_dw_b, f), 4)
    bias[:, 1] = np.tile(np.asarray(ksa_dw_b, f), 4)
    bias[:, 2] = np.tile(np.asarray(up_dw_b, f), 4)
    bias[:32, 3] = np.tile(np.asarray(kca_m1_b, f), 4)
    bias[:, 4] = np.tile(np.asarray(kca_m2_b, f), 4)
    bias[:100, 5] = np.tile(np.asarray(ksa_m1_b, f), 2)
    for sl in range(2):
        bias[sl * 32:sl * 32 + 25, 6] = np.asarray(ksa_m2_b, f)
    b_low = np.asarray(low_pw_w, f) @ np.asarray(low_dyn_b, f).reshape(CH) \
        + np.asarray(low_pw_b, f)
    bias[:, 7] = np.tile(b_low, 4)
    bias[:, 8] = np.tile(np.asarray(up_pw_b, f), 4)
    return dict(w_dw=w_dw.astype(bf), w_dw8=w_dw8.astype(f8),
                sel=sel.astype(bf),
                w_m1=w_m1.astype(bf), w_m2=w_m2.astype(bf),
                w_k1=w_k1.astype(bf), w_k2=w_k2.astype(bf),
                w_low=w_low.astype(bf), w_up=w_up.astype(bf),
                w_dyn=w_dyn, biases=bias)


def kernel(lower, upper, **wts):
    global LAST_EXEC_NS
    import ml_dtypes
    bf = ml_dtypes.bfloat16
    nc = _build_nc()
    wmap = _prep_weights(**wts)
    lp = np.pad(np.ascontiguousarray(np.asarray(lower, np.float32)),
                ((0, 0), (0, 0), (2, 2), (2, 2))).astype(bf)
    up = np.pad(np.ascontiguousarray(np.asarray(upper, np.float32)),
                ((0, 0), (0, 0), (2, 2), (2, 2))).astype(bf)

    def stripe(x, dt):
        # (32, 132, 260) -> (128 = strip*32+c, 132, 68), strips overlap by 4
        out = np.empty((128, HSH + 4, SW + 4), dt)
        for s in range(4):
            out[s * 32:(s + 1) * 32] = x[:, :, s * SW:s * SW + SW + 4]
        return out

    f8 = ml_dtypes.float8_e4m3
    in_maps = []
    for k in range(N_CORES):
        n, half = k // 2, k % 2
        m = dict(wmap)
        ls = lp[n, :, half * HSH:half * HSH + HSH + 4, :]
        us = up[n, :, half * HSH:half * HSH + HSH + 4, :]
        m["lower_sh"] = stripe(ls, bf)
        m["upper_sh"] = stripe(us, bf)

        def pad8(a):
            # one extra zero row for the (tap-row-4, zero) DR pseudo-pairs
            out = np.zeros((128, HSH + 5, SW + 4), f8)
            out[:, :HSH + 4] = a.astype(f8)
            return out

        m["lower8_sh"] = pad8(m["lower_sh"])
        m["upper8_sh"] = pad8(m["upper_sh"])
        in_maps.append(m)
    trace = os.environ.get("BASS_KERNEL_TRACE", "0") == "1"
    res = run_bass_kernel_spmd(nc, in_maps, core_ids=list(range(N_CORES)),
                               trace=trace)
    LAST_EXEC_NS = res.exec_time_ns
    low = np.empty((NB, CH, H, W), np.float32)
    upo = np.empty((NB, CH, H, W), np.float32)
    for k in range(N_CORES):
        n, half = k // 2, k % 2
        for s in range(4):
            low[n, :, half * HSH:(half + 1) * HSH, s * SW:(s + 1) * SW] = \
                res.results[k]["low_out"][s * 32:(s + 1) * 32]
            upo[n, :, half * HSH:(half + 1) * HSH, s * SW:(s + 1) * SW] = \
                res.results[k]["up_out"][s * 32:(s + 1) * 32]
    return low, upo



# revision 43
# speedup vs baseline: 1.0210x; 1.0210x over previous
"""CIKA conv block on 8 Trainium2 NeuronCores.

Sharding: pure data parallel. 8 shards = (batch n, H half). Each core gets a
zero-padded, W-strip-interleaved bf16 slice of `lower`/`upper` plus
replicated (host-preprocessed) weights, and computes its (32, 128, 256)
slice of both outputs (low, up).

On-chip layout: [128 partitions = 4 W-strips x 32 channels].  Depthwise 5x5
convs run on the TensorEngine as 25 diagonal-matmul taps accumulated in PSUM
(spatial shifts are free AP offsets into the padded SBUF plane).  1x1 convs
are block-diagonal matmuls (kron(I_strips, W^T)).  The dynamic (involution)
conv: a selector matmul replicates each KSA tap plane across the 32 channel
partitions into PSUM; one fused DVE scalar_tensor_tensor forms
m_t = (x_shift * w[c,t]) * ksa_rep in bf16; the tap sum and the following
1x1 are folded into one PSUM accumulation of W_low @ m_t over the 25 taps.
All matmul operands are bf16 (enables PE fast-weight-load); PSUM
accumulation stays fp32.
"""

import os
from contextlib import ExitStack

import numpy as np

import bass_rust
import concourse.bacc as bacc
import concourse.bass as bass
import concourse.mybir as mybir
import concourse.tile as tile
from concourse.bass_utils import run_bass_kernel_spmd

F32 = mybir.dt.float32
BF16 = mybir.dt.bfloat16
F16 = mybir.dt.float16
F8 = mybir.dt.float8e4
DR = mybir.MatmulPerfMode.DoubleRow
AF = mybir.ActivationFunctionType
ALU = mybir.AluOpType

KK = 5          # kernel size
CH = 32         # channels
NB, H, W = 4, 256, 256
N_CORES = 8
HSH = H // 2    # rows per core (one batch-half per core)
ROWS_T = 32     # output rows per on-chip tile
NT = HSH // ROWS_T
SW = 64         # strip width (W / 4)
TAPS = [(i, j) for i in range(KK) for j in range(KK)]

LAST_EXEC_NS = None


def _emit(ctx: ExitStack, tc: tile.TileContext, io):
    nc = tc.nc
    (lower_d, upper_d, lower8_d, upper8_d, wdw_d, wdw8_d, sel_d, wm1_d,
     wm2_d, wk1_d, wk2_d, wlow_d, wup_d, wdyn_d, bias_d, low_od, up_od) = io

    wpool = ctx.enter_context(tc.tile_pool(name="wts", bufs=1))
    inp = ctx.enter_context(tc.tile_pool(name="inp", bufs=2))
    work = ctx.enter_context(tc.tile_pool(name="work", bufs=2))
    outp = ctx.enter_context(tc.tile_pool(name="outp", bufs=2))
    ps_dw = ctx.enter_context(tc.tile_pool(name="psdw", bufs=2, space="PSUM"))
    ps_pw = ctx.enter_context(tc.tile_pool(name="pspw", bufs=2, space="PSUM"))
    ps_rep = ctx.enter_context(tc.tile_pool(name="psrep", bufs=2,
                                            space="PSUM"))

    # ---- DMA order tuned so tile-0 gate-conv can start ASAP ----
    # 1) tile-0 fp8 inputs + the weights the kca chain needs first
    low8_0 = inp.tile([128, ROWS_T + 5, SW + 4], F8, tag="low8_in")
    up8_0 = inp.tile([128, ROWS_T + 5, SW + 4], F8, tag="up8_in")
    nc.sync.dma_start(low8_0[:], lower8_d[:, 0:ROWS_T + 5, :])
    nc.sync.dma_start(up8_0[:], upper8_d[:, 0:ROWS_T + 5, :])
    # fp8 DoubleRow tap-pair weights for the two gate-path dw5 convs.
    # Vertical pairs (k-tile delta = row stride): per col j, row pairs
    # (0,1), (2,3), (4,zero) -> 15 pairs. [128, cv*30 + pair*2 + kt, 128]
    w_dw8 = wpool.tile([128, 60, 128], F8)
    nc.sync.dma_start(w_dw8[:], wdw8_d[:])
    w_m1 = wpool.tile([128, 32], BF16)
    nc.sync.dma_start(w_m1[:], wm1_d[:])
    w_m2 = wpool.tile([32, 128], BF16)
    nc.sync.dma_start(w_m2[:], wm2_d[:])
    bias = wpool.tile([128, 9], F32)
    nc.sync.dma_start(bias[:], bias_d[:])
    # 2) tile-0 bf16 inputs (needed mid-tile by the STT / up branch)
    low_0 = inp.tile([128, ROWS_T + 4, SW + 4], BF16, tag="low_in")
    up_0 = inp.tile([128, ROWS_T + 4, SW + 4], BF16, tag="up_in")
    nc.sync.dma_start(low_0[:], lower_d[:, 0:ROWS_T + 4, :])
    nc.sync.dma_start(up_0[:], upper_d[:, 0:ROWS_T + 4, :])
    # 3) remaining weights in order of first use
    w_k1 = wpool.tile([128, 100], BF16)
    nc.sync.dma_start(w_k1[:], wk1_d[:])
    w_k2 = wpool.tile([100, 64], BF16)
    nc.sync.dma_start(w_k2[:], wk2_d[:])
    sel = wpool.tile([128, 25, 128], BF16)
    nc.sync.dma_start(sel[:], sel_d[:])
    w_low = wpool.tile([128, 128], BF16)
    nc.sync.dma_start(w_low[:], wlow_d[:])
    wdyn = wpool.tile([128, 25], F32)
    nc.sync.dma_start(wdyn[:], wdyn_d[:])
    # w_dw holds only the up-branch dw5 taps (bf16 diagonal per tap)
    w_dw = wpool.tile([128, 25, 128], BF16)
    nc.sync.dma_start(w_dw[:], wdw_d[:])
    w_up = wpool.tile([128, 128], BF16)
    nc.sync.dma_start(w_up[:], wup_d[:])

    def bcol(idx, p=128):
        return bias[0:p, idx:idx + 1]

    # PE can encode only one sync wait per matmul (LDWEIGHTS struct limit).
    # Warm-up matmuls make PE observe every weight-DMA queue once, so real
    # matmuls transitively need no weight waits — just their rhs producer.
    # Split into two groups so the first real matmuls only wait on the
    # early (small) weight transfers.
    sc = ps_pw.tile([1, 1], F32, tag="pspw")
    for wap in (w_dw8[0:1, 0, 0:1], w_m1[0:1, 0:1], w_m2[0:1, 0:1]):
        nc.tensor.matmul(sc[:], wap, wap, start=True, stop=True)

    def late_warmups():
        sc2 = ps_pw.tile([1, 1], F32, tag="pspw")
        for wap in (w_k1[0:1, 0:1], w_k2[0:1, 0:1], sel[0:1, 0, 0:1],
                    w_low[0:1, 0:1], w_dw[0:1, 0, 0:1], w_up[0:1, 0:1]):
            nc.tensor.matmul(sc2[:], wap, wap, start=True, stop=True)

    add_ctr = [0]
    for it in range(NT):
        r0 = it * ROWS_T
        if it == 0:
            low_t, up_t, low8_t, up8_t = low_0, up_0, low8_0, up8_0
        else:
            low_t = inp.tile([128, ROWS_T + 4, SW + 4], BF16, tag="low_in")
            up_t = inp.tile([128, ROWS_T + 4, SW + 4], BF16, tag="up_in")
            low8_t = inp.tile([128, ROWS_T + 5, SW + 4], F8, tag="low8_in")
            up8_t = inp.tile([128, ROWS_T + 5, SW + 4], F8, tag="up8_in")
            # shards pre-striped on the host to [128 = strip*32+c, rows, 68]
            nc.sync.dma_start(low_t[:], lower_d[:, r0:r0 + ROWS_T + 4, :])
            nc.sync.dma_start(up_t[:], upper_d[:, r0:r0 + ROWS_T + 4, :])
            nc.sync.dma_start(low8_t[:], lower8_d[:, r0:r0 + ROWS_T + 5, :])
            nc.sync.dma_start(up8_t[:], upper8_d[:, r0:r0 + ROWS_T + 5, :])

        def dr_rhs(src8, q, i0, j):
            # [128, 2 (vertical tap-pair k-tiles, delta = row stride), 8, 64]
            base = src8[:, q * 8 + i0:q * 8 + i0 + 8, j:j + SW]
            raw = [list(d) for d in base.ap]
            return bass_rust.AP(
                base.tensor, base.offset,
                [raw[0], [SW + 4, 2], raw[1], raw[2]])

        # gate-path dw5: 15 fp8 DoubleRow vertical tap-pairs per q-chunk
        # (per col j: row pairs (0,1), (2,3), (4,zero))
        def dw5_dr(src8, cv, out_sb, bias_idx):
            for q in range(4):
                ps = ps_dw.tile([128, 8, SW], F32, tag="psdw")
                pp = 0
                for j in range(5):
                    for i0 in (0, 2, 4):
                        w8 = cv * 30 + pp * 2
                        nc.tensor.matmul(
                            ps[:], w_dw8[:, w8:w8 + 2, :],
                            dr_rhs(src8, q, i0, j),
                            start=(pp == 0), stop=(pp == 14), perf_mode=DR)
                        pp += 1
                nc.scalar.activation(out_sb[:, q * 8:(q + 1) * 8, :], ps[:],
                                     AF.Relu, bias=bcol(bias_idx))

        t_kca = work.tile([128, ROWS_T, SW], BF16, tag="t_kca")
        dw5_dr(low8_t, 0, t_kca, 0)
        if it == 0:
            late_warmups()

        # ---- KCA chain: 1x1 (32->8) relu, 1x1 (8->32) sigmoid ----
        m1o = work.tile([32, ROWS_T, SW], BF16, tag="m1o")
        for q in range(4):
            ps = ps_pw.tile([32, 8, SW], F32, tag="pspw")
            nc.tensor.matmul(ps[:], w_m1[:], t_kca[:, q * 8:(q + 1) * 8, :],
                             start=True, stop=True)
            nc.scalar.activation(m1o[:, q * 8:(q + 1) * 8, :], ps[:],
                                 AF.Relu, bias=bcol(3, 32))
        kca = work.tile([128, ROWS_T, SW], BF16, tag="kca")
        for q in range(4):
            ps = ps_pw.tile([128, 8, SW], F32, tag="pspw")
            nc.tensor.matmul(ps[:], w_m2[:], m1o[:, q * 8:(q + 1) * 8, :],
                             start=True, stop=True)
            nc.scalar.activation(kca[:, q * 8:(q + 1) * 8, :], ps[:],
                                 AF.Sigmoid, bias=bcol(4))

        # ---- KSA chain (strip pairs: K=64 -> M=100, then K=100 -> M=64) ----
        t_ksa = work.tile([128, ROWS_T, SW], BF16, tag="t_ksa")
        dw5_dr(up8_t, 1, t_ksa, 1)
        k1o = work.tile([100, 2, ROWS_T, SW], BF16, tag="k1o")
        for g in range(2):
            for q in range(4):
                ps = ps_pw.tile([100, 8, SW], F32, tag="pspw")
                nc.tensor.matmul(
                    ps[:], w_k1[g * 64:(g + 1) * 64, :],
                    t_ksa[g * 64:(g + 1) * 64, q * 8:(q + 1) * 8, :],
                    start=True, stop=True)
                nc.scalar.activation(k1o[:, g, q * 8:(q + 1) * 8, :], ps[:],
                                     AF.Relu, bias=bcol(5, 100))
        # ksa laid out [128 = strip*32 + tap, rows, cols] (slots 25-31 pad)
        ksa = work.tile([128, ROWS_T, SW], BF16, tag="ksa")
        for g in range(2):
            for q in range(4):
                ps = ps_pw.tile([64, 8, SW], F32, tag="pspw")
                nc.tensor.matmul(ps[:], w_k2[:],
                                 k1o[:, g, q * 8:(q + 1) * 8, :],
                                 start=True, stop=True)
                nc.scalar.activation(
                    ksa[64 * g:64 * (g + 1), q * 8:(q + 1) * 8, :], ps[:],
                    AF.Sigmoid, bias=bcol(6, 64))

        # ---- dynamic conv + low 1x1 ----
        # m_t = (lower_shift * w_dyn[c,t]) * ksa_rep[t].  The tap sum runs as
        # an fp16 ping-pong add chain on DVE/GpSimd (rep spans 2 PSUM banks so
        # each STT covers 16 rows); W_low is applied once to the summed taps.
        low_o = outp.tile([128, ROWS_T, SW], BF16, tag="low_o")

        def tadd(dst, a, b):
            # adds are all-SBUF: split between GpSimd (idle) and DVE
            if add_ctr[0] % 8 < 5:
                nc.gpsimd.tensor_add(dst[:], a[:], b[:])
            else:
                nc.vector.tensor_add(dst[:], a[:], b[:])
            add_ctr[0] += 1

        for hf in range(2):
            # binary-counter tree accumulation of the 25 fp16 tap products
            levels = [None] * 6
            for t, (i, j) in enumerate(TAPS):
                rep = ps_rep.tile([128, 16, SW], F32, tag="rep")
                for qq in range(2):
                    q = hf * 2 + qq
                    nc.tensor.matmul(rep[:, qq * 8:(qq + 1) * 8, :],
                                     sel[:, t, :],
                                     ksa[:, q * 8:(q + 1) * 8, :],
                                     start=True, stop=True)
                mt = work.tile([128, 16, SW], F16, tag="mt", bufs=3)
                nc.vector.scalar_tensor_tensor(
                    mt[:], low_t[:, hf * 16 + i:hf * 16 + i + 16, j:j + SW],
                    wdyn[:, t:t + 1], rep[:], ALU.mult, ALU.mult)
                cur, lvl = mt, 0
                while levels[lvl] is not None:
                    nxt = work.tile([128, 16, SW], F16, tag=f"bc{lvl}",
                                    name=f"bc{lvl}", bufs=3)
                    tadd(nxt, levels[lvl], cur)
                    levels[lvl] = None
                    cur, lvl = nxt, lvl + 1
                levels[lvl] = cur
            acc = None
            for lvl in range(6):
                if levels[lvl] is None:
                    continue
                if acc is None:
                    acc = levels[lvl]
                else:
                    nxt = work.tile([128, 16, SW], F16, tag=f"fm{lvl}",
                                    name=f"fm{lvl}", bufs=2)
                    tadd(nxt, acc, levels[lvl])
                    acc = nxt
            for qq in range(2):
                q = hf * 2 + qq
                ps = ps_pw.tile([128, 8, SW], F32, tag="pspw")
                nc.tensor.matmul(ps[:], w_low[:],
                                 acc[:, qq * 8:(qq + 1) * 8, :],
                                 start=True, stop=True)
                nc.scalar.activation(low_o[:, q * 8:(q + 1) * 8, :], ps[:],
                                     AF.Identity, bias=bcol(7))
        nc.sync.dma_start(low_od[:, r0:r0 + ROWS_T, :], low_o[:])

        # ---- up branch: dw5 + bias, gate by kca (fused on DVE), 1x1 ----
        gated = work.tile([128, ROWS_T, SW], BF16, tag="gated")
        for q in range(4):
            ps = ps_dw.tile([128, 8, SW], F32, tag="psdw")
            for t, (i, j) in enumerate(TAPS):
                nc.tensor.matmul(
                    ps[:], w_dw[:, t, :],
                    up_t[:, q * 8 + i:q * 8 + i + 8, j:j + SW],
                    start=(t == 0), stop=(t == 24))
            nc.vector.scalar_tensor_tensor(
                gated[:, q * 8:(q + 1) * 8, :], ps[:], bcol(2),
                kca[:, q * 8:(q + 1) * 8, :], ALU.add, ALU.mult)
        up_o = outp.tile([128, ROWS_T, SW], BF16, tag="up_o")
        for q in range(4):
            ps = ps_pw.tile([128, 8, SW], F32, tag="pspw")
            nc.tensor.matmul(ps[:], w_up[:], gated[:, q * 8:(q + 1) * 8, :],
                             start=True, stop=True)
            nc.scalar.activation(up_o[:, q * 8:(q + 1) * 8, :], ps[:],
                                 AF.Identity, bias=bcol(8))
        nc.sync.dma_start(up_od[:, r0:r0 + ROWS_T, :], up_o[:])


_NC_CACHE = {}


def _build_nc():
    if "nc" in _NC_CACHE:
        return _NC_CACHE["nc"]
    nc = bacc.Bacc("TRN2", target_bir_lowering=False)
    lower_d = nc.dram_tensor("lower_sh", (128, HSH + 4, SW + 4), BF16,
                             kind="ExternalInput")
    upper_d = nc.dram_tensor("upper_sh", (128, HSH + 4, SW + 4), BF16,
                             kind="ExternalInput")
    lower8_d = nc.dram_tensor("lower8_sh", (128, HSH + 5, SW + 4), F8,
                              kind="ExternalInput")
    upper8_d = nc.dram_tensor("upper8_sh", (128, HSH + 5, SW + 4), F8,
                              kind="ExternalInput")
    wdw_d = nc.dram_tensor("w_dw", (128, 25, 128), BF16, kind="ExternalInput")
    wdw8_d = nc.dram_tensor("w_dw8", (128, 60, 128), F8,
                            kind="ExternalInput")
    sel_d = nc.dram_tensor("sel", (128, 25, 128), BF16, kind="ExternalInput")
    wm1_d = nc.dram_tensor("w_m1", (128, 32), BF16, kind="ExternalInput")
    wm2_d = nc.dram_tensor("w_m2", (32, 128), BF16, kind="ExternalInput")
    wk1_d = nc.dram_tensor("w_k1", (128, 100), BF16, kind="ExternalInput")
    wk2_d = nc.dram_tensor("w_k2", (100, 64), BF16, kind="ExternalInput")
    wlow_d = nc.dram_tensor("w_low", (128, 128), BF16, kind="ExternalInput")
    wup_d = nc.dram_tensor("w_up", (128, 128), BF16, kind="ExternalInput")
    wdyn_d = nc.dram_tensor("w_dyn", (128, 25), F32, kind="ExternalInput")
    bias_d = nc.dram_tensor("biases", (128, 9), F32, kind="ExternalInput")
    low_od = nc.dram_tensor("low_out", (128, HSH, SW), BF16,
                            kind="ExternalOutput")
    up_od = nc.dram_tensor("up_out", (128, HSH, SW), BF16,
                           kind="ExternalOutput")
    io = (lower_d, upper_d, lower8_d, upper8_d, wdw_d, wdw8_d, sel_d, wm1_d,
          wm2_d, wk1_d, wk2_d, wlow_d, wup_d, wdyn_d, bias_d, low_od, up_od)
    with tile.TileContext(nc) as tc:
        with ExitStack() as ctx:
            _emit(ctx, tc, io)
    nc.compile()
    _NC_CACHE["nc"] = nc
    return nc


def _prep_weights(kca_dw_w, kca_dw_b, kca_m1_w, kca_m1_b, kca_m2_w, kca_m2_b,
                  ksa_dw_w, ksa_dw_b, ksa_m1_w, ksa_m1_b, ksa_m2_w, ksa_m2_b,
                  low_dyn_w, low_dyn_b, low_pw_w, low_pw_b,
                  up_dw_w, up_dw_b, up_pw_w, up_pw_b):
    f = np.float32
    import ml_dtypes
    bf = ml_dtypes.bfloat16
    f8 = ml_dtypes.float8_e4m3
    w_dw = np.zeros((128, 25, 128), f)
    ar = np.arange(128)
    w2 = np.asarray(up_dw_w, f).reshape(CH, 25)
    for t in range(25):
        w_dw[ar, t, ar] = np.tile(w2[:, t], 4)
    # DoubleRow fp8 vertical tap-pair diagonals for kca/ksa dw5:
    # per col j, row pairs (0,1), (2,3), (4,zero)
    w_dw8 = np.zeros((128, 60, 128), f)
    for cv, wt in enumerate([kca_dw_w, ksa_dw_w]):
        w3 = np.asarray(wt, f).reshape(CH, 5, 5)  # (c, i, j)
        pp = 0
        for j in range(5):
            for i0 in (0, 2, 4):
                for kt in range(2):
                    if i0 + kt < 5:
                        w_dw8[ar, cv * 30 + pp * 2 + kt, ar] = \
                            np.tile(w3[:, i0 + kt, j], 4)
                pp += 1
    sel = np.zeros((128, 25, 128), f)
    for s in range(4):
        for t in range(25):
            sel[s * 32 + t, t, s * 32:(s + 1) * 32] = 1.0
    i4, i2 = np.eye(4, dtype=f), np.eye(2, dtype=f)
    w_m1 = np.kron(i4, np.asarray(kca_m1_w, f).T)        # (128, 32)
    w_m2 = np.kron(i4, np.asarray(kca_m2_w, f).T)        # (32, 128)
    w_k1 = np.kron(i2, np.asarray(ksa_m1_w, f).T)        # (64, 100)
    w_k1 = np.vstack([w_k1, w_k1])                       # (128, 100) dup
    w_k2 = np.zeros((100, 64), f)                        # padded to 32-slots
    w2t = np.asarray(ksa_m2_w, f).T                      # (50, 25)
    for sl in range(2):
        w_k2[sl * 50:(sl + 1) * 50, sl * 32:sl * 32 + 25] = w2t
    w_low = np.kron(i4, np.asarray(low_pw_w, f).T)       # (128, 128)
    w_up = np.kron(i4, np.asarray(up_pw_w, f).T)         # (128, 128)
    w_dyn = np.tile(np.asarray(low_dyn_w, f).reshape(CH, 25), (4, 1))
    bias = np.zeros((128, 9), f)
    bias[:, 0] = np.tile(np.asarray(kca_dw_b, f), 4)
    bias[:, 1] = np.tile(np.asarray(ksa_dw_b, f), 4)
    bias[:, 2] = np.tile(np.asarray(up_dw_b, f), 4)
    bias[:32, 3] = np.tile(np.asarray(kca_m1_b, f), 4)
    bias[:, 4] = np.tile(np.asarray(kca_m2_b, f), 4)
    bias[:100, 5] = np.tile(np.asarray(ksa_m1_b, f), 2)
    for sl in range(2):
        bias[sl * 32:sl * 32 + 25, 6] = np.asarray(ksa_m2_b, f)
    b_low = np.asarray(low_pw_w, f) @ np.asarray(low_dyn_b, f).reshape(CH) \
        + np.asarray(low_pw_b, f)
    bias[:, 7] = np.tile(b_low, 4)
    bias[:, 8] = np.tile(np.asarray(up_pw_b, f), 4)
    return dict(w_dw=w_dw.astype(bf), w_dw8=w_dw8.astype(f8),
                sel=sel.astype(bf),
                w_m1=w_m1.astype(bf), w_m2=w_m2.astype(bf),
                w_k1=w_k1.astype(bf), w_k2=w_k2.astype(bf),
                w_low=w_low.astype(bf), w_up=w_up.astype(bf),
                w_dyn=w_dyn, biases=bias)


def kernel(lower, upper, **wts):
    global LAST_EXEC_NS
    import ml_dtypes
    bf = ml_dtypes.bfloat16
    nc = _build_nc()
    wmap = _prep_weights(**wts)
    lp = np.pad(np.ascontiguousarray(np.asarray(lower, np.float32)),
                ((0, 0), (0, 0), (2, 2), (2, 2))).astype(bf)
    up = np.pad(np.ascontiguousarray(np.asarray(upper, np.float32)),
                ((0, 0), (0, 0), (2, 2), (2, 2))).astype(bf)

    def stripe(x, dt):
        # (32, 132, 260) -> (128 = strip*32+c, 132, 68), strips overlap by 4
        out = np.empty((128, HSH + 4, SW + 4), dt)
        for s in range(4):
            out[s * 32:(s + 1) * 32] = x[:, :, s * SW:s * SW + SW + 4]
        return out

    f8 = ml_dtypes.float8_e4m3
    in_maps = []
    for k in range(N_CORES):
        n, half = k // 2, k % 2
        m = dict(wmap)
        ls = lp[n, :, half * HSH:half * HSH + HSH + 4, :]
        us = up[n, :, half * HSH:half * HSH + HSH + 4, :]
        m["lower_sh"] = stripe(ls, bf)
        m["upper_sh"] = stripe(us, bf)

        def pad8(a):
            # one extra zero row for the (tap-row-4, zero) DR pseudo-pairs
            out = np.zeros((128, HSH + 5, SW + 4), f8)
            out[:, :HSH + 4] = a.astype(f8)
            return out

        m["lower8_sh"] = pad8(m["lower_sh"])
        m["upper8_sh"] = pad8(m["upper_sh"])
        in_maps.append(m)
    trace = os.environ.get("BASS_KERNEL_TRACE", "0") == "1"
    res = run_bass_kernel_spmd(nc, in_maps, core_ids=list(range(N_CORES)),
                               trace=trace)
    LAST_EXEC_NS = res.exec_time_ns
    low = np.empty((NB, CH, H, W), np.float32)
    upo = np.empty((NB, CH, H, W), np.float32)
    for k in range(N_CORES):
        n, half = k // 2, k % 2
        for s in range(4):
            low[n, :, half * HSH:(half + 1) * HSH, s * SW:(s + 1) * SW] = \
                res.results[k]["low_out"][s * 32:(s + 1) * 32]
            upo[n, :, half * HSH:(half + 1) * HSH, s * SW:(s + 1) * SW] = \
                res.results[k]["up_out"][s * 32:(s + 1) * 32]
    return low, upo



# revision 45
# speedup vs baseline: 1.1614x; 1.1375x over previous
"""CIKA conv block on 8 Trainium2 NeuronCores.

Sharding: pure data parallel. 8 shards = (batch n, H half). Each core gets a
zero-padded, W-strip-interleaved bf16 slice of `lower`/`upper` plus
replicated (host-preprocessed) weights, and computes its (32, 128, 256)
slice of both outputs (low, up).

On-chip layout: [128 partitions = 4 W-strips x 32 channels].  Depthwise 5x5
convs run on the TensorEngine as 25 diagonal-matmul taps accumulated in PSUM
(spatial shifts are free AP offsets into the padded SBUF plane).  1x1 convs
are block-diagonal matmuls (kron(I_strips, W^T)).  The dynamic (involution)
conv: a selector matmul replicates each KSA tap plane across the 32 channel
partitions into PSUM; one fused DVE scalar_tensor_tensor forms
m_t = (x_shift * w[c,t]) * ksa_rep in bf16; the tap sum and the following
1x1 are folded into one PSUM accumulation of W_low @ m_t over the 25 taps.
All matmul operands are bf16 (enables PE fast-weight-load); PSUM
accumulation stays fp32.
"""

import os
from contextlib import ExitStack

import numpy as np

import bass_rust
import concourse.bacc as bacc
import concourse.bass as bass
import concourse.mybir as mybir
import concourse.tile as tile
from concourse.bass_utils import run_bass_kernel_spmd

F32 = mybir.dt.float32
BF16 = mybir.dt.bfloat16
F16 = mybir.dt.float16
F8 = mybir.dt.float8e4
DR = mybir.MatmulPerfMode.DoubleRow
AF = mybir.ActivationFunctionType
ALU = mybir.AluOpType

KK = 5          # kernel size
CH = 32         # channels
NB, H, W = 4, 256, 256
N_CORES = 8
HSH = H // 2    # rows per core (one batch-half per core)
ROWS_T = 32     # output rows per on-chip tile
NT = HSH // ROWS_T
SW = 64         # strip width (W / 4)
TAPS = [(i, j) for i in range(KK) for j in range(KK)]

LAST_EXEC_NS = None


def _emit(ctx: ExitStack, tc: tile.TileContext, io):
    nc = tc.nc
    (lower_d, upper_d, lower8_d, upper8_d, wdw_d, wdw8_d, sel_d, wm1_d,
     wm2_d, wk1_d, wk2_d, wlow_d, wup_d, wdyn_d, bias_d, low_od, up_od) = io

    wpool = ctx.enter_context(tc.tile_pool(name="wts", bufs=1))
    inp = ctx.enter_context(tc.tile_pool(name="inp", bufs=2))
    work = ctx.enter_context(tc.tile_pool(name="work", bufs=2))
    outp = ctx.enter_context(tc.tile_pool(name="outp", bufs=2))
    ps_dw = ctx.enter_context(tc.tile_pool(name="psdw", bufs=2, space="PSUM"))
    ps_pw = ctx.enter_context(tc.tile_pool(name="pspw", bufs=2, space="PSUM"))
    ps_rep = ctx.enter_context(tc.tile_pool(name="psrep", bufs=2,
                                            space="PSUM"))

    # ---- DMA order tuned so tile-0 gate-conv can start ASAP ----
    # 1) tile-0 fp8 inputs + the weights the kca chain needs first
    low8_0 = inp.tile([128, ROWS_T + 5, SW + 4], F8, tag="low8_in")
    up8_0 = inp.tile([128, ROWS_T + 5, SW + 4], F8, tag="up8_in")
    nc.sync.dma_start(low8_0[:], lower8_d[:, 0:ROWS_T + 5, :])
    nc.sync.dma_start(up8_0[:], upper8_d[:, 0:ROWS_T + 5, :])
    # fp8 DoubleRow tap-pair weights for the two gate-path dw5 convs.
    # Vertical pairs (k-tile delta = row stride): per col j, row pairs
    # (0,1), (2,3), (4,zero) -> 15 pairs. [128, cv*30 + pair*2 + kt, 128]
    w_dw8 = wpool.tile([128, 60, 128], F8)
    nc.sync.dma_start(w_dw8[:], wdw8_d[:])
    w_m1 = wpool.tile([128, 32], BF16)
    nc.sync.dma_start(w_m1[:], wm1_d[:])
    w_m2 = wpool.tile([32, 128], BF16)
    nc.sync.dma_start(w_m2[:], wm2_d[:])
    bias = wpool.tile([128, 9], F32)
    nc.sync.dma_start(bias[:], bias_d[:])
    # 2) tile-0 bf16 inputs (needed mid-tile by the STT / up branch)
    low_0 = inp.tile([128, ROWS_T + 4, SW + 4], BF16, tag="low_in")
    up_0 = inp.tile([128, ROWS_T + 4, SW + 4], BF16, tag="up_in")
    nc.sync.dma_start(low_0[:], lower_d[:, 0:ROWS_T + 4, :])
    nc.sync.dma_start(up_0[:], upper_d[:, 0:ROWS_T + 4, :])
    # 3) remaining weights in order of first use
    w_k1 = wpool.tile([128, 100], BF16)
    nc.sync.dma_start(w_k1[:], wk1_d[:])
    w_k2 = wpool.tile([100, 64], BF16)
    nc.sync.dma_start(w_k2[:], wk2_d[:])
    sel = wpool.tile([128, 25, 128], BF16)
    nc.sync.dma_start(sel[:], sel_d[:])
    w_low = wpool.tile([128, 128], BF16)
    nc.sync.dma_start(w_low[:], wlow_d[:])
    wdyn = wpool.tile([128, 25], F32)
    nc.sync.dma_start(wdyn[:], wdyn_d[:])
    # w_dw holds only the up-branch dw5 taps (bf16 diagonal per tap)
    w_dw = wpool.tile([128, 25, 128], BF16)
    nc.sync.dma_start(w_dw[:], wdw_d[:])
    w_up = wpool.tile([128, 128], BF16)
    nc.sync.dma_start(w_up[:], wup_d[:])

    def bcol(idx, p=128):
        return bias[0:p, idx:idx + 1]

    # PE can encode only one sync wait per matmul (LDWEIGHTS struct limit).
    # Warm-up matmuls make PE observe every weight-DMA queue once, so real
    # matmuls transitively need no weight waits — just their rhs producer.
    # Split into two groups so the first real matmuls only wait on the
    # early (small) weight transfers.
    sc = ps_pw.tile([1, 1], F32, tag="pspw")
    for wap in (w_dw8[0:1, 0, 0:1], w_m1[0:1, 0:1], w_m2[0:1, 0:1]):
        nc.tensor.matmul(sc[:], wap, wap, start=True, stop=True)

    def late_warmups():
        sc2 = ps_pw.tile([1, 1], F32, tag="pspw")
        for wap in (w_k1[0:1, 0:1], w_k2[0:1, 0:1], sel[0:1, 0, 0:1],
                    w_low[0:1, 0:1], w_dw[0:1, 0, 0:1], w_up[0:1, 0:1]):
            nc.tensor.matmul(sc2[:], wap, wap, start=True, stop=True)

    add_ctr = [0]
    pending = [None]
    for it in range(NT):
        r0 = it * ROWS_T
        if it == 0:
            low_t, up_t, low8_t, up8_t = low_0, up_0, low8_0, up8_0
        else:
            low_t = inp.tile([128, ROWS_T + 4, SW + 4], BF16, tag="low_in")
            up_t = inp.tile([128, ROWS_T + 4, SW + 4], BF16, tag="up_in")
            low8_t = inp.tile([128, ROWS_T + 5, SW + 4], F8, tag="low8_in")
            up8_t = inp.tile([128, ROWS_T + 5, SW + 4], F8, tag="up8_in")
            # shards pre-striped on the host to [128 = strip*32+c, rows, 68]
            nc.sync.dma_start(low_t[:], lower_d[:, r0:r0 + ROWS_T + 4, :])
            nc.sync.dma_start(up_t[:], upper_d[:, r0:r0 + ROWS_T + 4, :])
            nc.sync.dma_start(low8_t[:], lower8_d[:, r0:r0 + ROWS_T + 5, :])
            nc.sync.dma_start(up8_t[:], upper8_d[:, r0:r0 + ROWS_T + 5, :])

        def dr_rhs(src8, q, i0, j):
            # [128, 2 (vertical tap-pair k-tiles, delta = row stride), 8, 64]
            base = src8[:, q * 8 + i0:q * 8 + i0 + 8, j:j + SW]
            raw = [list(d) for d in base.ap]
            return bass_rust.AP(
                base.tensor, base.offset,
                [raw[0], [SW + 4, 2], raw[1], raw[2]])

        # gate-path dw5: 15 fp8 DoubleRow vertical tap-pairs per q-chunk
        # (per col j: row pairs (0,1), (2,3), (4,zero))
        def dw5_dr(src8, cv, out_sb, bias_idx):
            for q in range(4):
                ps = ps_dw.tile([128, 8, SW], F32, tag="psdw")
                pp = 0
                for j in range(5):
                    for i0 in (0, 2, 4):
                        w8 = cv * 30 + pp * 2
                        nc.tensor.matmul(
                            ps[:], w_dw8[:, w8:w8 + 2, :],
                            dr_rhs(src8, q, i0, j),
                            start=(pp == 0), stop=(pp == 14), perf_mode=DR)
                        pp += 1
                nc.scalar.activation(out_sb[:, q * 8:(q + 1) * 8, :], ps[:],
                                     AF.Relu, bias=bcol(bias_idx))

        t_kca = work.tile([128, ROWS_T, SW], BF16, tag="t_kca")
        dw5_dr(low8_t, 0, t_kca, 0)
        if it == 0:
            late_warmups()

        # ---- KCA chain: 1x1 (32->8) relu, 1x1 (8->32) sigmoid ----
        m1o = work.tile([32, ROWS_T, SW], BF16, tag="m1o")
        for q in range(4):
            ps = ps_pw.tile([32, 8, SW], F32, tag="pspw")
            nc.tensor.matmul(ps[:], w_m1[:], t_kca[:, q * 8:(q + 1) * 8, :],
                             start=True, stop=True)
            nc.scalar.activation(m1o[:, q * 8:(q + 1) * 8, :], ps[:],
                                 AF.Relu, bias=bcol(3, 32))
        kca = work.tile([128, ROWS_T, SW], BF16, tag="kca")
        for q in range(4):
            ps = ps_pw.tile([128, 8, SW], F32, tag="pspw")
            nc.tensor.matmul(ps[:], w_m2[:], m1o[:, q * 8:(q + 1) * 8, :],
                             start=True, stop=True)
            nc.scalar.activation(kca[:, q * 8:(q + 1) * 8, :], ps[:],
                                 AF.Sigmoid, bias=bcol(4))

        # ---- KSA chain (strip pairs: K=64 -> M=100, then K=100 -> M=64) ----
        t_ksa = work.tile([128, ROWS_T, SW], BF16, tag="t_ksa")
        dw5_dr(up8_t, 1, t_ksa, 1)
        k1o = work.tile([100, 2, ROWS_T, SW], BF16, tag="k1o")
        for g in range(2):
            for q in range(4):
                ps = ps_pw.tile([100, 8, SW], F32, tag="pspw")
                nc.tensor.matmul(
                    ps[:], w_k1[g * 64:(g + 1) * 64, :],
                    t_ksa[g * 64:(g + 1) * 64, q * 8:(q + 1) * 8, :],
                    start=True, stop=True)
                nc.scalar.activation(k1o[:, g, q * 8:(q + 1) * 8, :], ps[:],
                                     AF.Relu, bias=bcol(5, 100))
        # ksa laid out [128 = strip*32 + tap, rows, cols] (slots 25-31 pad)
        ksa = work.tile([128, ROWS_T, SW], BF16, tag="ksa")
        for g in range(2):
            for q in range(4):
                ps = ps_pw.tile([64, 8, SW], F32, tag="pspw")
                nc.tensor.matmul(ps[:], w_k2[:],
                                 k1o[:, g, q * 8:(q + 1) * 8, :],
                                 start=True, stop=True)
                nc.scalar.activation(
                    ksa[64 * g:64 * (g + 1), q * 8:(q + 1) * 8, :], ps[:],
                    AF.Sigmoid, bias=bcol(6, 64))

        # ---- deferred W_low + low store of the PREVIOUS tile (its fp16
        # tap-sum tree finished while this tile's gates ran, so the PE never
        # blocks on it) ----
        if pending[0] is not None:
            pending[0]()
            pending[0] = None

        # ---- up branch as a generator: its PE matmuls are pulled in between
        # the sel replication pairs below, keeping the PE fed while the DVE
        # paces the dynamic-conv products ----
        gated = work.tile([128, ROWS_T, SW], BF16, tag="gated")
        up_o = outp.tile([128, ROWS_T, SW], BF16, tag="up_o")

        def up_branch_ops(up_t=up_t, gated=gated, up_o=up_o, r0=r0):
            for q in range(4):
                ps = ps_dw.tile([128, 8, SW], F32, tag="psdw", name="psu")
                for t in range(25):
                    i, j = TAPS[t]
                    nc.tensor.matmul(
                        ps[:], w_dw[:, t, :],
                        up_t[:, q * 8 + i:q * 8 + i + 8, j:j + SW],
                        start=(t == 0), stop=(t == 24))
                    yield
                nc.vector.scalar_tensor_tensor(
                    gated[:, q * 8:(q + 1) * 8, :], ps[:], bcol(2),
                    kca[:, q * 8:(q + 1) * 8, :], ALU.add, ALU.mult)
                ps2 = ps_pw.tile([128, 8, SW], F32, tag="pspw", name="psu2")
                nc.tensor.matmul(ps2[:], w_up[:],
                                 gated[:, q * 8:(q + 1) * 8, :],
                                 start=True, stop=True)
                nc.scalar.activation(up_o[:, q * 8:(q + 1) * 8, :], ps2[:],
                                     AF.Identity, bias=bcol(8))
                yield
            nc.sync.dma_start(up_od[:, r0:r0 + ROWS_T, :], up_o[:])

        filler = up_branch_ops()

        def pull(n):
            for _ in range(n):
                try:
                    next(filler)
                except StopIteration:
                    return

        # ---- dynamic conv products ----
        # m_t = (lower_shift * w_dyn[c,t]) * ksa_rep[t]; the tap sum runs as
        # an fp16 binary-counter add tree on GpSimd/DVE (rep spans 2 PSUM
        # banks so each STT covers 16 rows).
        low_o = outp.tile([128, ROWS_T, SW], BF16, tag="low_o")

        def tadd(dst, a, b):
            # adds are all-SBUF: split between GpSimd (idle) and DVE
            if add_ctr[0] % 8 < 5:
                nc.gpsimd.tensor_add(dst[:], a[:], b[:])
            else:
                nc.vector.tensor_add(dst[:], a[:], b[:])
            add_ctr[0] += 1

        acc_hf = []
        for hf in range(2):
            # binary-counter tree accumulation of the 25 fp16 tap products
            levels = [None] * 6
            for t, (i, j) in enumerate(TAPS):
                rep = ps_rep.tile([128, 16, SW], F32, tag="rep")
                for qq in range(2):
                    q = hf * 2 + qq
                    nc.tensor.matmul(rep[:, qq * 8:(qq + 1) * 8, :],
                                     sel[:, t, :],
                                     ksa[:, q * 8:(q + 1) * 8, :],
                                     start=True, stop=True)
                pull(2)
                mt = work.tile([128, 16, SW], F16, tag="mt", bufs=3)
                nc.vector.scalar_tensor_tensor(
                    mt[:], low_t[:, hf * 16 + i:hf * 16 + i + 16, j:j + SW],
                    wdyn[:, t:t + 1], rep[:], ALU.mult, ALU.mult)
                cur, lvl = mt, 0
                while levels[lvl] is not None:
                    nxt = work.tile([128, 16, SW], F16, tag=f"bc{lvl}",
                                    name=f"bc{lvl}", bufs=3)
                    tadd(nxt, levels[lvl], cur)
                    levels[lvl] = None
                    cur, lvl = nxt, lvl + 1
                levels[lvl] = cur
            acc = None
            for lvl in range(6):
                if levels[lvl] is None:
                    continue
                if acc is None:
                    acc = levels[lvl]
                else:
                    nxt = work.tile([128, 16, SW], F16, tag=f"fm{lvl}",
                                    name=f"fm{lvl}", bufs=2)
                    tadd(nxt, acc, levels[lvl])
                    acc = nxt
            acc_hf.append(acc)
        pull(120)

        def mk_wlow(acc_hf=acc_hf, low_o=low_o, r0=r0):
            def f():
                for hf in range(2):
                    for qq in range(2):
                        q = hf * 2 + qq
                        ps = ps_pw.tile([128, 8, SW], F32, tag="pspw",
                                        name="psw")
                        nc.tensor.matmul(
                            ps[:], w_low[:],
                            acc_hf[hf][:, qq * 8:(qq + 1) * 8, :],
                            start=True, stop=True)
                        nc.scalar.activation(low_o[:, q * 8:(q + 1) * 8, :],
                                             ps[:], AF.Identity, bias=bcol(7))
                nc.sync.dma_start(low_od[:, r0:r0 + ROWS_T, :], low_o[:])
            return f

        pending[0] = mk_wlow()
    pending[0]()


_NC_CACHE = {}


def _build_nc():
    if "nc" in _NC_CACHE:
        return _NC_CACHE["nc"]
    nc = bacc.Bacc("TRN2", target_bir_lowering=False)
    lower_d = nc.dram_tensor("lower_sh", (128, HSH + 4, SW + 4), BF16,
                             kind="ExternalInput")
    upper_d = nc.dram_tensor("upper_sh", (128, HSH + 4, SW + 4), BF16,
                             kind="ExternalInput")
    lower8_d = nc.dram_tensor("lower8_sh", (128, HSH + 5, SW + 4), F8,
                              kind="ExternalInput")
    upper8_d = nc.dram_tensor("upper8_sh", (128, HSH + 5, SW + 4), F8,
                              kind="ExternalInput")
    wdw_d = nc.dram_tensor("w_dw", (128, 25, 128), BF16, kind="ExternalInput")
    wdw8_d = nc.dram_tensor("w_dw8", (128, 60, 128), F8,
                            kind="ExternalInput")
    sel_d = nc.dram_tensor("sel", (128, 25, 128), BF16, kind="ExternalInput")
    wm1_d = nc.dram_tensor("w_m1", (128, 32), BF16, kind="ExternalInput")
    wm2_d = nc.dram_tensor("w_m2", (32, 128), BF16, kind="ExternalInput")
    wk1_d = nc.dram_tensor("w_k1", (128, 100), BF16, kind="ExternalInput")
    wk2_d = nc.dram_tensor("w_k2", (100, 64), BF16, kind="ExternalInput")
    wlow_d = nc.dram_tensor("w_low", (128, 128), BF16, kind="ExternalInput")
    wup_d = nc.dram_tensor("w_up", (128, 128), BF16, kind="ExternalInput")
    wdyn_d = nc.dram_tensor("w_dyn", (128, 25), F32, kind="ExternalInput")
    bias_d = nc.dram_tensor("biases", (128, 9), F32, kind="ExternalInput")
    low_od = nc.dram_tensor("low_out", (128, HSH, SW), BF16,
                            kind="ExternalOutput")
    up_od = nc.dram_tensor("up_out", (128, HSH, SW), BF16,
                           kind="ExternalOutput")
    io = (lower_d, upper_d, lower8_d, upper8_d, wdw_d, wdw8_d, sel_d, wm1_d,
          wm2_d, wk1_d, wk2_d, wlow_d, wup_d, wdyn_d, bias_d, low_od, up_od)
    with tile.TileContext(nc) as tc:
        with ExitStack() as ctx:
            _emit(ctx, tc, io)
    nc.compile()
    _NC_CACHE["nc"] = nc
    return nc


def _prep_weights(kca_dw_w, kca_dw_b, kca_m1_w, kca_m1_b, kca_m2_w, kca_m2_b,
                  ksa_dw_w, ksa_dw_b, ksa_m1_w, ksa_m1_b, ksa_m2_w, ksa_m2_b,
                  low_dyn_w, low_dyn_b, low_pw_w, low_pw_b,
                  up_dw_w, up_dw_b, up_pw_w, up_pw_b):
    f = np.float32
    import ml_dtypes
    bf = ml_dtypes.bfloat16
    f8 = ml_dtypes.float8_e4m3
    w_dw = np.zeros((128, 25, 128), f)
    ar = np.arange(128)
    w2 = np.asarray(up_dw_w, f).reshape(CH, 25)
    for t in range(25):
        w_dw[ar, t, ar] = np.tile(w2[:, t], 4)
    # DoubleRow fp8 vertical tap-pair diagonals for kca/ksa dw5:
    # per col j, row pairs (0,1), (2,3), (4,zero)
    w_dw8 = np.zeros((128, 60, 128), f)
    for cv, wt in enumerate([kca_dw_w, ksa_dw_w]):
        w3 = np.asarray(wt, f).reshape(CH, 5, 5)  # (c, i, j)
        pp = 0
        for j in range(5):
            for i0 in (0, 2, 4):
                for kt in range(2):
                    if i0 + kt < 5:
                        w_dw8[ar, cv * 30 + pp * 2 + kt, ar] = \
                            np.tile(w3[:, i0 + kt, j], 4)
                pp += 1
    sel = np.zeros((128, 25, 128), f)
    for s in range(4):
        for t in range(25):
            sel[s * 32 + t, t, s * 32:(s + 1) * 32] = 1.0
    i4, i2 = np.eye(4, dtype=f), np.eye(2, dtype=f)
    w_m1 = np.kron(i4, np.asarray(kca_m1_w, f).T)        # (128, 32)
    w_m2 = np.kron(i4, np.asarray(kca_m2_w, f).T)        # (32, 128)
    w_k1 = np.kron(i2, np.asarray(ksa_m1_w, f).T)        # (64, 100)
    w_k1 = np.vstack([w_k1, w_k1])                       # (128, 100) dup
    w_k2 = np.zeros((100, 64), f)                        # padded to 32-slots
    w2t = np.asarray(ksa_m2_w, f).T                      # (50, 25)
    for sl in range(2):
        w_k2[sl * 50:(sl + 1) * 50, sl * 32:sl * 32 + 25] = w2t
    w_low = np.kron(i4, np.asarray(low_pw_w, f).T)       # (128, 128)
    w_up = np.kron(i4, np.asarray(up_pw_w, f).T)         # (128, 128)
    w_dyn = np.tile(np.asarray(low_dyn_w, f).reshape(CH, 25), (4, 1))
    bias = np.zeros((128, 9), f)
    bias[:, 0] = np.tile(np.asarray(kca_dw_b, f), 4)
    bias[:, 1] = np.tile(np.asarray(ksa_dw_b, f), 4)
    bias[:, 2] = np.tile(np.asarray(up_dw_b, f), 4)
    bias[:32, 3] = np.tile(np.asarray(kca_m1_b, f), 4)
    bias[:, 4] = np.tile(np.asarray(kca_m2_b, f), 4)
    bias[:100, 5] = np.tile(np.asarray(ksa_m1_b, f), 2)
    for sl in range(2):
        bias[sl * 32:sl * 32 + 25, 6] = np.asarray(ksa_m2_b, f)
    b_low = np.asarray(low_pw_w, f) @ np.asarray(low_dyn_b, f).reshape(CH) \
        + np.asarray(low_pw_b, f)
    bias[:, 7] = np.tile(b_low, 4)
    bias[:, 8] = np.tile(np.asarray(up_pw_b, f), 4)
    return dict(w_dw=w_dw.astype(bf), w_dw8=w_dw8.astype(f8),
                sel=sel.astype(bf),
                w_m1=w_m1.astype(bf), w_m2=w_m2.astype(bf),
                w_k1=w_k1.astype(bf), w_k2=w_k2.astype(bf),
                w_low=w_low.astype(bf), w_up=w_up.astype(bf),
                w_dyn=w_dyn, biases=bias)


def kernel(lower, upper, **wts):
    global LAST_EXEC_NS
    import ml_dtypes
    bf = ml_dtypes.bfloat16
    nc = _build_nc()
    wmap = _prep_weights(**wts)
    lp = np.pad(np.ascontiguousarray(np.asarray(lower, np.float32)),
                ((0, 0), (0, 0), (2, 2), (2, 2))).astype(bf)
    up = np.pad(np.ascontiguousarray(np.asarray(upper, np.float32)),
                ((0, 0), (0, 0), (2, 2), (2, 2))).astype(bf)

    def stripe(x, dt):
        # (32, 132, 260) -> (128 = strip*32+c, 132, 68), strips overlap by 4
        out = np.empty((128, HSH + 4, SW + 4), dt)
        for s in range(4):
            out[s * 32:(s + 1) * 32] = x[:, :, s * SW:s * SW + SW + 4]
        return out

    f8 = ml_dtypes.float8_e4m3
    in_maps = []
    for k in range(N_CORES):
        n, half = k // 2, k % 2
        m = dict(wmap)
        ls = lp[n, :, half * HSH:half * HSH + HSH + 4, :]
        us = up[n, :, half * HSH:half * HSH + HSH + 4, :]
        m["lower_sh"] = stripe(ls, bf)
        m["upper_sh"] = stripe(us, bf)

        def pad8(a):
            # one extra zero row for the (tap-row-4, zero) DR pseudo-pairs
            out = np.zeros((128, HSH + 5, SW + 4), f8)
            out[:, :HSH + 4] = a.astype(f8)
            return out

        m["lower8_sh"] = pad8(m["lower_sh"])
        m["upper8_sh"] = pad8(m["upper_sh"])
        in_maps.append(m)
    trace = os.environ.get("BASS_KERNEL_TRACE", "0") == "1"
    res = run_bass_kernel_spmd(nc, in_maps, core_ids=list(range(N_CORES)),
                               trace=trace)
    LAST_EXEC_NS = res.exec_time_ns
    low = np.empty((NB, CH, H, W), np.float32)
    upo = np.empty((NB, CH, H, W), np.float32)
    for k in range(N_CORES):
        n, half = k // 2, k % 2
        for s in range(4):
            low[n, :, half * HSH:(half + 1) * HSH, s * SW:(s + 1) * SW] = \
                res.results[k]["low_out"][s * 32:(s + 1) * 32]
            upo[n, :, half * HSH:(half + 1) * HSH, s * SW:(s + 1) * SW] = \
                res.results[k]["up_out"][s * 32:(s + 1) * 32]
    return low, upo



# revision 47
# speedup vs baseline: 1.1831x; 1.0187x over previous
"""CIKA conv block on 8 Trainium2 NeuronCores.

Sharding: pure data parallel. 8 shards = (batch n, H half). Each core gets a
zero-padded, W-strip-interleaved bf16 slice of `lower`/`upper` plus
replicated (host-preprocessed) weights, and computes its (32, 128, 256)
slice of both outputs (low, up).

On-chip layout: [128 partitions = 4 W-strips x 32 channels].  Depthwise 5x5
convs run on the TensorEngine as 25 diagonal-matmul taps accumulated in PSUM
(spatial shifts are free AP offsets into the padded SBUF plane).  1x1 convs
are block-diagonal matmuls (kron(I_strips, W^T)).  The dynamic (involution)
conv: a selector matmul replicates each KSA tap plane across the 32 channel
partitions into PSUM; one fused DVE scalar_tensor_tensor forms
m_t = (x_shift * w[c,t]) * ksa_rep in bf16; the tap sum and the following
1x1 are folded into one PSUM accumulation of W_low @ m_t over the 25 taps.
All matmul operands are bf16 (enables PE fast-weight-load); PSUM
accumulation stays fp32.
"""

import os
from contextlib import ExitStack

import numpy as np

import bass_rust
import concourse.bacc as bacc
import concourse.bass as bass
import concourse.mybir as mybir
import concourse.tile as tile
from concourse.bass_utils import run_bass_kernel_spmd

F32 = mybir.dt.float32
BF16 = mybir.dt.bfloat16
F16 = mybir.dt.float16
F8 = mybir.dt.float8e4
DR = mybir.MatmulPerfMode.DoubleRow
AF = mybir.ActivationFunctionType
ALU = mybir.AluOpType

KK = 5          # kernel size
CH = 32         # channels
NB, H, W = 4, 256, 256
N_CORES = 8
HSH = H // 2    # rows per core (one batch-half per core)
ROWS_T = 32     # output rows per on-chip tile
NT = HSH // ROWS_T
SW = 64         # strip width (W / 4)
TAPS = [(i, j) for i in range(KK) for j in range(KK)]

LAST_EXEC_NS = None


def _emit(ctx: ExitStack, tc: tile.TileContext, io):
    nc = tc.nc
    (lower_d, upper_d, lower8_d, upper8_d, wdw_d, wdw8_d, sel_d, wm1_d,
     wm2_d, wk1_d, wk2_d, wlow_d, wup_d, wdyn_d, bias_d, low_od, up_od) = io

    wpool = ctx.enter_context(tc.tile_pool(name="wts", bufs=1))
    inp = ctx.enter_context(tc.tile_pool(name="inp", bufs=2))
    work = ctx.enter_context(tc.tile_pool(name="work", bufs=2))
    outp = ctx.enter_context(tc.tile_pool(name="outp", bufs=2))
    ps_dw = ctx.enter_context(tc.tile_pool(name="psdw", bufs=2, space="PSUM"))
    ps_pw = ctx.enter_context(tc.tile_pool(name="pspw", bufs=2, space="PSUM"))
    ps_rep = ctx.enter_context(tc.tile_pool(name="psrep", bufs=2,
                                            space="PSUM"))

    # ---- DMA order tuned so tile-0 gate-conv can start ASAP ----
    # 1) tile-0 fp8 inputs + the weights the kca chain needs first
    low8_0 = inp.tile([128, ROWS_T + 5, SW + 4], F8, tag="low8_in")
    up8_0 = inp.tile([128, ROWS_T + 5, SW + 4], F8, tag="up8_in")
    nc.sync.dma_start(low8_0[:], lower8_d[:, 0:ROWS_T + 5, :])
    nc.sync.dma_start(up8_0[:], upper8_d[:, 0:ROWS_T + 5, :])
    # fp8 DoubleRow tap-pair weights for the two gate-path dw5 convs.
    # Vertical pairs (k-tile delta = row stride): per col j, row pairs
    # (0,1), (2,3), (4,zero) -> 15 pairs. [128, cv*30 + pair*2 + kt, 128]
    w_dw8 = wpool.tile([128, 60, 128], F8)
    nc.sync.dma_start(w_dw8[:], wdw8_d[:])
    w_m1 = wpool.tile([128, 32], BF16)
    nc.sync.dma_start(w_m1[:], wm1_d[:])
    w_m2 = wpool.tile([32, 128], BF16)
    nc.sync.dma_start(w_m2[:], wm2_d[:])
    bias = wpool.tile([128, 9], F32)
    nc.sync.dma_start(bias[:], bias_d[:])
    # 2) tile-0 bf16 inputs (needed mid-tile by the STT / up branch)
    low_0 = inp.tile([128, ROWS_T + 4, SW + 4], BF16, tag="low_in")
    up_0 = inp.tile([128, ROWS_T + 4, SW + 4], BF16, tag="up_in")
    nc.sync.dma_start(low_0[:], lower_d[:, 0:ROWS_T + 4, :])
    nc.sync.dma_start(up_0[:], upper_d[:, 0:ROWS_T + 4, :])
    # 3) remaining weights in order of first use
    w_k1 = wpool.tile([128, 100], BF16)
    nc.sync.dma_start(w_k1[:], wk1_d[:])
    w_k2 = wpool.tile([100, 64], BF16)
    nc.sync.dma_start(w_k2[:], wk2_d[:])
    sel = wpool.tile([128, 25, 128], BF16)
    nc.sync.dma_start(sel[:], sel_d[:])
    w_low = wpool.tile([128, 128], BF16)
    nc.sync.dma_start(w_low[:], wlow_d[:])
    wdyn = wpool.tile([128, 25], F32)
    nc.sync.dma_start(wdyn[:], wdyn_d[:])
    # w_dw holds only the up-branch dw5 taps (bf16 diagonal per tap)
    w_dw = wpool.tile([128, 25, 128], BF16)
    nc.sync.dma_start(w_dw[:], wdw_d[:])
    w_up = wpool.tile([128, 128], BF16)
    nc.sync.dma_start(w_up[:], wup_d[:])

    def bcol(idx, p=128):
        return bias[0:p, idx:idx + 1]

    # PE can encode only one sync wait per matmul (LDWEIGHTS struct limit).
    # Warm-up matmuls make PE observe every weight-DMA queue once, so real
    # matmuls transitively need no weight waits — just their rhs producer.
    # Split into two groups so the first real matmuls only wait on the
    # early (small) weight transfers.
    sc = ps_pw.tile([1, 1], F32, tag="pspw")
    for wap in (w_dw8[0:1, 0, 0:1], w_m1[0:1, 0:1], w_m2[0:1, 0:1]):
        nc.tensor.matmul(sc[:], wap, wap, start=True, stop=True)

    def late_warmups():
        sc2 = ps_pw.tile([1, 1], F32, tag="pspw")
        for wap in (w_k1[0:1, 0:1], w_k2[0:1, 0:1], sel[0:1, 0, 0:1],
                    w_low[0:1, 0:1], w_dw[0:1, 0, 0:1], w_up[0:1, 0:1]):
            nc.tensor.matmul(sc2[:], wap, wap, start=True, stop=True)

    add_ctr = [0]
    pending = [None]
    for it in range(NT):
        r0 = it * ROWS_T
        if it == 0:
            low_t, up_t, low8_t, up8_t = low_0, up_0, low8_0, up8_0
        else:
            low_t = inp.tile([128, ROWS_T + 4, SW + 4], BF16, tag="low_in")
            up_t = inp.tile([128, ROWS_T + 4, SW + 4], BF16, tag="up_in")
            low8_t = inp.tile([128, ROWS_T + 5, SW + 4], F8, tag="low8_in")
            up8_t = inp.tile([128, ROWS_T + 5, SW + 4], F8, tag="up8_in")
            # shards pre-striped on the host to [128 = strip*32+c, rows, 68]
            nc.sync.dma_start(low_t[:], lower_d[:, r0:r0 + ROWS_T + 4, :])
            nc.sync.dma_start(up_t[:], upper_d[:, r0:r0 + ROWS_T + 4, :])
            nc.sync.dma_start(low8_t[:], lower8_d[:, r0:r0 + ROWS_T + 5, :])
            nc.sync.dma_start(up8_t[:], upper8_d[:, r0:r0 + ROWS_T + 5, :])

        def dr_rhs(src8, q, i0, j):
            # [128, 2 (vertical tap-pair k-tiles, delta = row stride), 8, 64]
            base = src8[:, q * 8 + i0:q * 8 + i0 + 8, j:j + SW]
            raw = [list(d) for d in base.ap]
            return bass_rust.AP(
                base.tensor, base.offset,
                [raw[0], [SW + 4, 2], raw[1], raw[2]])

        # gate-path dw5: 15 fp8 DoubleRow vertical tap-pairs per q-chunk
        # (per col j: row pairs (0,1), (2,3), (4,zero))
        def dw5_dr(src8, cv, out_sb, bias_idx):
            for q in range(4):
                ps = ps_dw.tile([128, 8, SW], F32, tag="psdw")
                pp = 0
                for j in range(5):
                    for i0 in (0, 2, 4):
                        w8 = cv * 30 + pp * 2
                        nc.tensor.matmul(
                            ps[:], w_dw8[:, w8:w8 + 2, :],
                            dr_rhs(src8, q, i0, j),
                            start=(pp == 0), stop=(pp == 14), perf_mode=DR)
                        pp += 1
                nc.scalar.activation(out_sb[:, q * 8:(q + 1) * 8, :], ps[:],
                                     AF.Relu, bias=bcol(bias_idx))

        t_kca = work.tile([128, ROWS_T, SW], BF16, tag="t_kca")
        dw5_dr(low8_t, 0, t_kca, 0)
        if it == 0:
            late_warmups()

        # ---- KCA chain: 1x1 (32->8) relu, 1x1 (8->32) sigmoid ----
        m1o = work.tile([32, ROWS_T, SW], BF16, tag="m1o")
        for q in range(4):
            ps = ps_pw.tile([32, 8, SW], F32, tag="pspw")
            nc.tensor.matmul(ps[:], w_m1[:], t_kca[:, q * 8:(q + 1) * 8, :],
                             start=True, stop=True)
            nc.scalar.activation(m1o[:, q * 8:(q + 1) * 8, :], ps[:],
                                 AF.Relu, bias=bcol(3, 32))
        kca = work.tile([128, ROWS_T, SW], BF16, tag="kca")
        for q in range(4):
            ps = ps_pw.tile([128, 8, SW], F32, tag="pspw")
            nc.tensor.matmul(ps[:], w_m2[:], m1o[:, q * 8:(q + 1) * 8, :],
                             start=True, stop=True)
            nc.scalar.activation(kca[:, q * 8:(q + 1) * 8, :], ps[:],
                                 AF.Sigmoid, bias=bcol(4))

        # ---- KSA chain (strip pairs: K=64 -> M=100, then K=100 -> M=64) ----
        t_ksa = work.tile([128, ROWS_T, SW], BF16, tag="t_ksa")
        dw5_dr(up8_t, 1, t_ksa, 1)
        k1o = work.tile([100, 2, ROWS_T, SW], BF16, tag="k1o")
        for g in range(2):
            for q in range(4):
                ps = ps_pw.tile([100, 8, SW], F32, tag="pspw")
                nc.tensor.matmul(
                    ps[:], w_k1[g * 64:(g + 1) * 64, :],
                    t_ksa[g * 64:(g + 1) * 64, q * 8:(q + 1) * 8, :],
                    start=True, stop=True)
                nc.scalar.activation(k1o[:, g, q * 8:(q + 1) * 8, :], ps[:],
                                     AF.Relu, bias=bcol(5, 100))
        # ksa laid out [128 = strip*32 + tap, rows, cols] (slots 25-31 pad)
        ksa = work.tile([128, ROWS_T, SW], BF16, tag="ksa")
        for g in range(2):
            for q in range(4):
                ps = ps_pw.tile([64, 8, SW], F32, tag="pspw")
                nc.tensor.matmul(ps[:], w_k2[:],
                                 k1o[:, g, q * 8:(q + 1) * 8, :],
                                 start=True, stop=True)
                nc.scalar.activation(
                    ksa[64 * g:64 * (g + 1), q * 8:(q + 1) * 8, :], ps[:],
                    AF.Sigmoid, bias=bcol(6, 64))

        # ---- deferred W_low + low store of the PREVIOUS tile (its fp16
        # tap-sum tree finished while this tile's gates ran, so the PE never
        # blocks on it) ----
        if pending[0] is not None:
            pending[0]()
            pending[0] = None

        # ---- up branch as a generator: its PE matmuls are pulled in between
        # the sel replication pairs below, keeping the PE fed while the DVE
        # paces the dynamic-conv products ----
        gated = work.tile([128, ROWS_T, SW], BF16, tag="gated")
        up_o = outp.tile([128, ROWS_T, SW], BF16, tag="up_o")

        def up_branch_ops(up_t=up_t, gated=gated, up_o=up_o, r0=r0):
            # the 1x1 for chunk q is deferred into chunk q+1's tap stream so
            # the PE never waits on the gated STT that is still queued on DVE
            deferred = [None]

            def fin(q):
                def f():
                    ps2 = ps_pw.tile([128, 8, SW], F32, tag="pspw",
                                     name="psu2")
                    nc.tensor.matmul(ps2[:], w_up[:],
                                     gated[:, q * 8:(q + 1) * 8, :],
                                     start=True, stop=True)
                    nc.scalar.activation(up_o[:, q * 8:(q + 1) * 8, :],
                                         ps2[:], AF.Identity, bias=bcol(8))
                return f

            for q in range(4):
                ps = ps_dw.tile([128, 8, SW], F32, tag="psdw", name="psu")
                for t in range(25):
                    i, j = TAPS[t]
                    nc.tensor.matmul(
                        ps[:], w_dw[:, t, :],
                        up_t[:, q * 8 + i:q * 8 + i + 8, j:j + SW],
                        start=(t == 0), stop=(t == 24))
                    if t == 12 and deferred[0] is not None:
                        deferred[0]()
                        deferred[0] = None
                    yield
                nc.vector.scalar_tensor_tensor(
                    gated[:, q * 8:(q + 1) * 8, :], ps[:], bcol(2),
                    kca[:, q * 8:(q + 1) * 8, :], ALU.add, ALU.mult)
                deferred[0] = fin(q)
                yield
            deferred[0]()
            nc.sync.dma_start(up_od[:, r0:r0 + ROWS_T, :], up_o[:])

        filler = up_branch_ops()

        def pull(n):
            for _ in range(n):
                try:
                    next(filler)
                except StopIteration:
                    return

        # ---- dynamic conv products ----
        # m_t = (lower_shift * w_dyn[c,t]) * ksa_rep[t]; the tap sum runs as
        # an fp16 binary-counter add tree on GpSimd/DVE (rep spans 2 PSUM
        # banks so each STT covers 16 rows).
        low_o = outp.tile([128, ROWS_T, SW], BF16, tag="low_o")

        def tadd(dst, a, b):
            # adds are all-SBUF: split between GpSimd (idle) and DVE
            if add_ctr[0] % 2 == 0:
                nc.gpsimd.tensor_add(dst[:], a[:], b[:])
            else:
                nc.vector.tensor_add(dst[:], a[:], b[:])
            add_ctr[0] += 1

        acc_hf = []
        for hf in range(2):
            # binary-counter tree accumulation of the 25 fp16 tap products
            levels = [None] * 6
            for t, (i, j) in enumerate(TAPS):
                rep = ps_rep.tile([128, 16, SW], F32, tag="rep")
                for qq in range(2):
                    q = hf * 2 + qq
                    nc.tensor.matmul(rep[:, qq * 8:(qq + 1) * 8, :],
                                     sel[:, t, :],
                                     ksa[:, q * 8:(q + 1) * 8, :],
                                     start=True, stop=True)
                pull(2)
                mt = work.tile([128, 16, SW], F16, tag="mt", bufs=8)
                nc.vector.scalar_tensor_tensor(
                    mt[:], low_t[:, hf * 16 + i:hf * 16 + i + 16, j:j + SW],
                    wdyn[:, t:t + 1], rep[:], ALU.mult, ALU.mult)
                cur, lvl = mt, 0
                while levels[lvl] is not None:
                    nxt = work.tile([128, 16, SW], F16, tag=f"bc{lvl}",
                                    name=f"bc{lvl}", bufs=4)
                    tadd(nxt, levels[lvl], cur)
                    levels[lvl] = None
                    cur, lvl = nxt, lvl + 1
                levels[lvl] = cur
            acc = None
            for lvl in range(6):
                if levels[lvl] is None:
                    continue
                if acc is None:
                    acc = levels[lvl]
                else:
                    nxt = work.tile([128, 16, SW], F16, tag=f"fm{lvl}",
                                    name=f"fm{lvl}", bufs=2)
                    tadd(nxt, acc, levels[lvl])
                    acc = nxt
            acc_hf.append(acc)
        pull(120)

        def mk_wlow(acc_hf=acc_hf, low_o=low_o, r0=r0):
            def f():
                for hf in range(2):
                    for qq in range(2):
                        q = hf * 2 + qq
                        ps = ps_pw.tile([128, 8, SW], F32, tag="pspw",
                                        name="psw")
                        nc.tensor.matmul(
                            ps[:], w_low[:],
                            acc_hf[hf][:, qq * 8:(qq + 1) * 8, :],
                            start=True, stop=True)
                        nc.scalar.activation(low_o[:, q * 8:(q + 1) * 8, :],
                                             ps[:], AF.Identity, bias=bcol(7))
                nc.sync.dma_start(low_od[:, r0:r0 + ROWS_T, :], low_o[:])
            return f

        pending[0] = mk_wlow()
    pending[0]()


_NC_CACHE = {}


def _build_nc():
    if "nc" in _NC_CACHE:
        return _NC_CACHE["nc"]
    nc = bacc.Bacc("TRN2", target_bir_lowering=False)
    lower_d = nc.dram_tensor("lower_sh", (128, HSH + 4, SW + 4), BF16,
                             kind="ExternalInput")
    upper_d = nc.dram_tensor("upper_sh", (128, HSH + 4, SW + 4), BF16,
                             kind="ExternalInput")
    lower8_d = nc.dram_tensor("lower8_sh", (128, HSH + 5, SW + 4), F8,
                              kind="ExternalInput")
    upper8_d = nc.dram_tensor("upper8_sh", (128, HSH + 5, SW + 4), F8,
                              kind="ExternalInput")
    wdw_d = nc.dram_tensor("w_dw", (128, 25, 128), BF16, kind="ExternalInput")
    wdw8_d = nc.dram_tensor("w_dw8", (128, 60, 128), F8,
                            kind="ExternalInput")
    sel_d = nc.dram_tensor("sel", (128, 25, 128), BF16, kind="ExternalInput")
    wm1_d = nc.dram_tensor("w_m1", (128, 32), BF16, kind="ExternalInput")
    wm2_d = nc.dram_tensor("w_m2", (32, 128), BF16, kind="ExternalInput")
    wk1_d = nc.dram_tensor("w_k1", (128, 100), BF16, kind="ExternalInput")
    wk2_d = nc.dram_tensor("w_k2", (100, 64), BF16, kind="ExternalInput")
    wlow_d = nc.dram_tensor("w_low", (128, 128), BF16, kind="ExternalInput")
    wup_d = nc.dram_tensor("w_up", (128, 128), BF16, kind="ExternalInput")
    wdyn_d = nc.dram_tensor("w_dyn", (128, 25), F32, kind="ExternalInput")
    bias_d = nc.dram_tensor("biases", (128, 9), F32, kind="ExternalInput")
    low_od = nc.dram_tensor("low_out", (128, HSH, SW), BF16,
                            kind="ExternalOutput")
    up_od = nc.dram_tensor("up_out", (128, HSH, SW), BF16,
                           kind="ExternalOutput")
    io = (lower_d, upper_d, lower8_d, upper8_d, wdw_d, wdw8_d, sel_d, wm1_d,
          wm2_d, wk1_d, wk2_d, wlow_d, wup_d, wdyn_d, bias_d, low_od, up_od)
    with tile.TileContext(nc) as tc:
        with ExitStack() as ctx:
            _emit(ctx, tc, io)
    nc.compile()
    _NC_CACHE["nc"] = nc
    return nc


def _prep_weights(kca_dw_w, kca_dw_b, kca_m1_w, kca_m1_b, kca_m2_w, kca_m2_b,
                  ksa_dw_w, ksa_dw_b, ksa_m1_w, ksa_m1_b, ksa_m2_w, ksa_m2_b,
                  low_dyn_w, low_dyn_b, low_pw_w, low_pw_b,
                  up_dw_w, up_dw_b, up_pw_w, up_pw_b):
    f = np.float32
    import ml_dtypes
    bf = ml_dtypes.bfloat16
    f8 = ml_dtypes.float8_e4m3
    w_dw = np.zeros((128, 25, 128), f)
    ar = np.arange(128)
    w2 = np.asarray(up_dw_w, f).reshape(CH, 25)
    for t in range(25):
        w_dw[ar, t, ar] = np.tile(w2[:, t], 4)
    # DoubleRow fp8 vertical tap-pair diagonals for kca/ksa dw5:
    # per col j, row pairs (0,1), (2,3), (4,zero)
    w_dw8 = np.zeros((128, 60, 128), f)
    for cv, wt in enumerate([kca_dw_w, ksa_dw_w]):
        w3 = np.asarray(wt, f).reshape(CH, 5, 5)  # (c, i, j)
        pp = 0
        for j in range(5):
            for i0 in (0, 2, 4):
                for kt in range(2):
                    if i0 + kt < 5:
                        w_dw8[ar, cv * 30 + pp * 2 + kt, ar] = \
                            np.tile(w3[:, i0 + kt, j], 4)
                pp += 1
    sel = np.zeros((128, 25, 128), f)
    for s in range(4):
        for t in range(25):
            sel[s * 32 + t, t, s * 32:(s + 1) * 32] = 1.0
    i4, i2 = np.eye(4, dtype=f), np.eye(2, dtype=f)
    w_m1 = np.kron(i4, np.asarray(kca_m1_w, f).T)        # (128, 32)
    w_m2 = np.kron(i4, np.asarray(kca_m2_w, f).T)        # (32, 128)
    w_k1 = np.kron(i2, np.asarray(ksa_m1_w, f).T)        # (64, 100)
    w_k1 = np.vstack([w_k1, w_k1])                       # (128, 100) dup
    w_k2 = np.zeros((100, 64), f)                        # padded to 32-slots
    w2t = np.asarray(ksa_m2_w, f).T                      # (50, 25)
    for sl in range(2):
        w_k2[sl * 50:(sl + 1) * 50, sl * 32:sl * 32 + 25] = w2t
    w_low = np.kron(i4, np.asarray(low_pw_w, f).T)       # (128, 128)
    w_up = np.kron(i4, np.asarray(up_pw_w, f).T)         # (128, 128)
    w_dyn = np.tile(np.asarray(low_dyn_w, f).reshape(CH, 25), (4, 1))
    bias = np.zeros((128, 9), f)
    bias[:, 0] = np.tile(np.asarray(kca_dw_b, f), 4)
    bias[:, 1] = np.tile(np.asarray(ksa_dw_b, f), 4)
    bias[:, 2] = np.tile(np.asarray(up_dw_b, f), 4)
    bias[:32, 3] = np.tile(np.asarray(kca_m1_b, f), 4)
    bias[:, 4] = np.tile(np.asarray(kca_m2_b, f), 4)
    bias[:100, 5] = np.tile(np.asarray(ksa_m1_b, f), 2)
    for sl in range(2):
        bias[sl * 32:sl * 32 + 25, 6] = np.asarray(ksa_m2_b, f)
    b_low = np.asarray(low_pw_w, f) @ np.asarray(low_dyn_b, f).reshape(CH) \
        + np.asarray(low_pw_b, f)
    bias[:, 7] = np.tile(b_low, 4)
    bias[:, 8] = np.tile(np.asarray(up_pw_b, f), 4)
    return dict(w_dw=w_dw.astype(bf), w_dw8=w_dw8.astype(f8),
                sel=sel.astype(bf),
                w_m1=w_m1.astype(bf), w_m2=w_m2.astype(bf),
                w_k1=w_k1.astype(bf), w_k2=w_k2.astype(bf),
                w_low=w_low.astype(bf), w_up=w_up.astype(bf),
                w_dyn=w_dyn, biases=bias)


def kernel(lower, upper, **wts):
    global LAST_EXEC_NS
    import ml_dtypes
    bf = ml_dtypes.bfloat16
    nc = _build_nc()
    wmap = _prep_weights(**wts)
    lp = np.pad(np.ascontiguousarray(np.asarray(lower, np.float32)),
                ((0, 0), (0, 0), (2, 2), (2, 2))).astype(bf)
    up = np.pad(np.ascontiguousarray(np.asarray(upper, np.float32)),
                ((0, 0), (0, 0), (2, 2), (2, 2))).astype(bf)

    def stripe(x, dt):
        # (32, 132, 260) -> (128 = strip*32+c, 132, 68), strips overlap by 4
        out = np.empty((128, HSH + 4, SW + 4), dt)
        for s in range(4):
            out[s * 32:(s + 1) * 32] = x[:, :, s * SW:s * SW + SW + 4]
        return out

    f8 = ml_dtypes.float8_e4m3
    in_maps = []
    for k in range(N_CORES):
        n, half = k // 2, k % 2
        m = dict(wmap)
        ls = lp[n, :, half * HSH:half * HSH + HSH + 4, :]
        us = up[n, :, half * HSH:half * HSH + HSH + 4, :]
        m["lower_sh"] = stripe(ls, bf)
        m["upper_sh"] = stripe(us, bf)

        def pad8(a):
            # one extra zero row for the (tap-row-4, zero) DR pseudo-pairs
            out = np.zeros((128, HSH + 5, SW + 4), f8)
            out[:, :HSH + 4] = a.astype(f8)
            return out

        m["lower8_sh"] = pad8(m["lower_sh"])
        m["upper8_sh"] = pad8(m["upper_sh"])
        in_maps.append(m)
    trace = os.environ.get("BASS_KERNEL_TRACE", "0") == "1"
    res = run_bass_kernel_spmd(nc, in_maps, core_ids=list(range(N_CORES)),
                               trace=trace)
    LAST_EXEC_NS = res.exec_time_ns
    low = np.empty((NB, CH, H, W), np.float32)
    upo = np.empty((NB, CH, H, W), np.float32)
    for k in range(N_CORES):
        n, half = k // 2, k % 2
        for s in range(4):
            low[n, :, half * HSH:(half + 1) * HSH, s * SW:(s + 1) * SW] = \
                res.results[k]["low_out"][s * 32:(s + 1) * 32]
            upo[n, :, half * HSH:(half + 1) * HSH, s * SW:(s + 1) * SW] = \
                res.results[k]["up_out"][s * 32:(s + 1) * 32]
    return low, upo

